# revision 13
# baseline (speedup 1.0000x reference)
"""Distributed Trainium2 kernel for a single attention head.

Problem: x:[8,2048,1024] f32, w_q/w_k/w_v:[1024,64] f32
  q,k,v = x@w ; scores = (q k^T)/sqrt(1024) causal-masked; out = softmax(scores)@v

Sharding: data-parallel over batch B=8 across the 8 NeuronCores (one batch
element per core, weights replicated, no collectives).

Per-core dataflow (T=2048, C=1024, H=64):
  - host ships x^T pre-tiled [128, chunk, c, 512] in bf16, packed w_qk / w_v
    (bf16), a 0/1 triangular mask tile, and identities for transposes.
  - ~10 warm-up matmuls on a scratch tile run while the input DMA streams so
    the PE HAM clock-gate reaches 8/8 before real work arrives.
  - all x/weight input DMAs ride the sync HWDGE queue (large transfers,
    chunk-0 split fine for early start); q/k partition-duplication and output
    DMAs ride the gpsimd SWDGE queue so they never queue behind the x stream.
  - projections with weights stationary (bf16): qkT [128,T] (q rows 0:64,
    k rows 64:128), vT [64,T]; q/k copied to SBUF and duplicated onto both
    partition halves so score pairs can run 2x row-packed on the PE.
  - scores computed TRANSPOSED per s-tile: S[s,t] = kT_slice.T @ qT (K=64),
    two s-tiles concurrently in PE row-groups 0/1; diagonal s-tiles only
    compute the columns t >= 128*rel that survive the causal mask.
  - exp on ScalarE with scale=1/32 folded in (|scores|<~2, no max needed),
    one [128,1024] activation per pair, output bf16; the ACT table set is
    pre-loaded by a dummy exp at t~0.
  - causal: diagonal 128x128 blocks multiplied by a 0/1 lower-triangle mask
    on VectorE after the exp (keeps the PE free of mask matmuls).
  - PV: out^T[h,t] accumulated over s-tiles with lhsT = [v | 1] so row 64 of
    the accumulator is the softmax denominator (fused row-sum).
  - epilogue: TensorE transpose back to [t,h], reciprocal-multiply on
    VectorE, one combined [512,64] DMA out per chunk.
  - next-chunk projections are emission-interleaved between attention pairs
    so the PE stream stays dense.
"""

import os
import sys

import numpy as np

for p in ("/opt/trn_rl_repo",):
    if p not in sys.path and os.path.isdir(p):
        sys.path.insert(0, p)

import ml_dtypes  # noqa: E402

B, T, C, H = 8, 2048, 1024, 64
N_CORES = 8
TCH = 512                  # t-chunk (columns per PSUM bank of f32)
N_CHUNK = T // TCH         # 4
N_CT = C // 128            # 8 contraction tiles
SCALE = float(C) ** -0.5   # 1/32
N_WARM = 10                # PE warm-up matmuls

_CACHE = {}


def _build():
    """Build + compile the SPMD Bass graph (same graph on all 8 cores)."""
    import concourse.bass as bass
    import concourse.mybir as mybir
    import concourse.tile as tile
    from concourse import bacc

    f32 = mybir.dt.float32
    bf16 = mybir.dt.bfloat16
    EXP = mybir.ActivationFunctionType.Exp

    nc = bacc.Bacc(
        "TRN2", target_bir_lowering=False, debug=False, num_devices=N_CORES
    )

    # host ships x^T pre-tiled: [128, N_CHUNK * N_CT * TCH] laid out
    # [chunk][c-tile][t] per partition so each chunk is one contiguous DMA.
    xT_d = nc.dram_tensor("xT", [128, N_CHUNK * N_CT * TCH], bf16, kind="ExternalInput")
    wqk_d = nc.dram_tensor("wqk", [128, N_CT * 128], bf16, kind="ExternalInput")
    wv_d = nc.dram_tensor("wv", [128, N_CT * H], bf16, kind="ExternalInput")
    mask_d = nc.dram_tensor("mask01", [128, 128], bf16, kind="ExternalInput")  # 0/1, keep s<=t
    idf_d = nc.dram_tensor("idf", [128, 128], f32, kind="ExternalInput")
    idb_d = nc.dram_tensor("idb", [128, 128], bf16, kind="ExternalInput")
    out_d = nc.dram_tensor("out", [T, H], f32, kind="ExternalOutput")

    with tile.TileContext(nc) as tc:
        with (
            tc.tile_pool(name="const", bufs=1) as constp,
            tc.tile_pool(name="xTp", bufs=1) as xTp,
            tc.tile_pool(name="qkp", bufs=1) as qkp,
            tc.tile_pool(name="q2p", bufs=2) as q2p,
            tc.tile_pool(name="vTp", bufs=2) as vTp,
            tc.tile_pool(name="v1p", bufs=1) as v1p,
            tc.tile_pool(name="exp", bufs=4) as expp,
            tc.tile_pool(name="epi", bufs=2) as epip,
            tc.tile_pool(name="Sp", bufs=2, space="PSUM") as Sp,
            tc.tile_pool(name="accp", bufs=1, space="PSUM") as accp,
            tc.tile_pool(name="miscp", bufs=3, space="PSUM") as miscp,
        ):
            # ---- PE warm-up scratch ----
            warm_sb = constp.tile([128, 256], bf16, tag="warm_sb", name="warm_sb")
            nc.vector.memset(warm_sb[:], 0.0)
            warm_act = constp.tile([128, 8], bf16, tag="warm_act", name="warm_act")
            warm_ps = miscp.tile([128, 256], f32, tag="misc", name="warm_ps")
            for i in range(N_WARM):
                nc.tensor.matmul(
                    warm_ps[:, :],
                    warm_sb[:, 0:128],
                    warm_sb[:, :],
                    start=True,
                    stop=True,
                    skip_group_check=True,
                )

            # ---- input DMAs: weights + x chunks, split across the two HWDGE
            # queues (sync + scalar) so issue overhead parallelizes and all
            # of chunk 0 sits at the head of both queues. ----
            wqk_t = constp.tile([128, N_CT, 128], bf16, tag="wqk", name="wqk_t")
            nc.sync.dma_start(
                out=wqk_t[:], in_=wqk_d[:].rearrange("p (n m) -> p n m", n=N_CT)
            )
            xt = {}
            for t in range(N_CHUNK):
                xt[t] = xTp.tile([128, N_CT, TCH], bf16, tag=f"x{t}", name=f"x{t}")
            xT_v = xT_d[:].rearrange("p (t n m) -> p t n m", t=N_CHUNK, n=N_CT)
            for h in range(4):  # chunk 0 in 4 pieces of 2 c-tiles
                eng = nc.sync if h < 2 else nc.scalar
                eng.dma_start(
                    out=xt[0][:, 2 * h : 2 * h + 2, :], in_=xT_v[:, 0, 2 * h : 2 * h + 2, :]
                )
            wv_t = constp.tile([128, N_CT, H], bf16, tag="wv", name="wv_t")
            nc.scalar.dma_start(
                out=wv_t[:], in_=wv_d[:].rearrange("p (n m) -> p n m", n=N_CT)
            )
            for t, eng in ((1, nc.scalar), (2, nc.sync), (3, nc.scalar)):
                eng.dma_start(out=xt[t][:], in_=xT_v[:, t, :, :])

            # dummy exp AFTER the scalar-queue DMA issues: forces the ACT
            # table-set load to overlap the input-DMA phase without delaying
            # the x stream behind the ~2.7us table load
            nc.scalar.activation(warm_act[:], warm_sb[:, 0:8], EXP, scale=1.0)

            # ---- small constants on the gpsimd SWDGE queue ----
            mask_t = constp.tile([128, 128], bf16, tag="mask", name="mask_t")
            nc.gpsimd.dma_start(out=mask_t[:], in_=mask_d[:])
            idb_t = constp.tile([128, 128], bf16, tag="idb", name="idb_t")
            nc.gpsimd.dma_start(out=idb_t[:], in_=idb_d[:])
            idf_t = constp.tile([128, 128], f32, tag="idf", name="idf_t")
            nc.gpsimd.dma_start(out=idf_t[:], in_=idf_d[:])

            qk2 = {}   # [128, TCH] bf16: qT duplicated on both partition halves
            kk2 = {}   # [128, TCH] bf16: kT duplicated on both partition halves
            v1 = {}    # [128, 65] bf16 per s-tile: [v | 1]

            def proj_steps(tch):
                """Emission thunks for chunk `tch`: (qk steps, v steps)."""
                qk_steps = []
                v_steps = []
                state = {}

                def qk_mm(c):
                    def f():
                        if c == 0:
                            state["S"] = miscp.tile(
                                [128, TCH], f32, tag="misc", name=f"Sqk{tch}"
                            )
                        nc.tensor.matmul(
                            state["S"][:, :],
                            wqk_t[:, c, :],
                            xt[tch][:, c, :],
                            start=(c == 0),
                            stop=(c == N_CT - 1),
                            skip_group_check=True,
                        )
                    return f

                def qk_out():
                    S = state["S"]
                    q2 = q2p.tile([128, TCH], bf16, tag="q2", name=f"q2_{tch}")
                    k2 = qkp.tile([128, TCH], bf16, tag=f"k2_{tch}", name=f"k2_{tch}")
                    nc.vector.tensor_copy(k2[0:64, :], S[64:128, :])
                    nc.vector.tensor_copy(q2[0:64, :], S[0:64, :])
                    # chunks 0-1 run their scores unpacked on partition half
                    # 0 (the ~2.5us SWDGE duplication round-trip would sit on
                    # their critical path); their q never needs duplicating.
                    # k still does (used by chunks 2-3 in row-packed pairs).
                    if tch > 1:
                        nc.gpsimd.dma_start(out=q2[64:128, :], in_=q2[0:64, :])
                    nc.gpsimd.dma_start(out=k2[64:128, :], in_=k2[0:64, :])
                    qk2[tch] = q2
                    kk2[tch] = k2

                def v_mm(c):
                    def f():
                        if c == 0:
                            state["Pv"] = miscp.tile(
                                [64, TCH], f32, tag="misc", name=f"Pv{tch}"
                            )
                        nc.tensor.matmul(
                            state["Pv"][:, :],
                            wv_t[:, c, :],
                            xt[tch][:, c, :],
                            start=(c == 0),
                            stop=(c == N_CT - 1),
                            skip_group_check=True,
                        )
                    return f

                def v_out():
                    vTt = vTp.tile([64, TCH], bf16, tag="vT", name=f"vT{tch}")
                    nc.vector.tensor_copy(vTt[:], state["Pv"][:, :])
                    state["vT"] = vTt

                def v1_build(i):
                    def f():
                        j = 4 * tch + i
                        Pt = miscp.tile([128, H], bf16, tag="misc", name=f"Pt{j}")
                        nc.tensor.transpose(
                            Pt[:, :],
                            state["vT"][:, 128 * i : 128 * (i + 1)],
                            idb_t[0:64, 0:64],
                        )
                        v1t = v1p.tile([128, 65], bf16, tag=f"v1_{j}", name=f"v1_{j}")
                        nc.vector.tensor_copy(v1t[:, 0:64], Pt[:, :])
                        nc.vector.memset(v1t[:, 64:65], 1.0)
                        v1[j] = v1t
                    return f

                for c in range(N_CT):
                    qk_steps.append(qk_mm(c))
                qk_steps.append(qk_out)
                for c in range(N_CT):
                    v_steps.append(v_mm(c))
                v_steps.append(v_out)
                for i in range(4):
                    v_steps.append(v1_build(i))
                return qk_steps, v_steps

            def emit_scores_exp(tch, jp, unpacked):
                """Scores matmuls + exp for pair (jp, jp+1); returns (ext, los)."""
                S2 = Sp.tile([128, 2 * TCH], f32, tag="S", name=f"S{tch}_{jp}")
                los = {}
                for jj in range(2):
                    j = jp + jj
                    rel = j - 4 * tch
                    lo = 128 * max(0, rel)
                    los[jj] = lo
                    half = slice(0, 64) if unpacked else slice(64 * jj, 64 * (jj + 1))
                    ksl = kk2[j // 4][half, 128 * (j % 4) : 128 * (j % 4 + 1)]
                    nc.tensor.matmul(
                        S2[:, TCH * jj + lo : TCH * (jj + 1)],
                        ksl,
                        qk2[tch][half, lo:TCH],
                        start=True,
                        stop=True,
                        skip_group_check=True,
                    )
                ext = expp.tile([128, 2 * TCH], bf16, tag="ex", name=f"ex{tch}_{jp}")
                nc.scalar.activation(ext[:], S2[:], EXP, scale=SCALE)
                # causal 0/1 mask on the diagonal 128x128 blocks (VectorE)
                for jj in range(2):
                    j = jp + jj
                    if j - 4 * tch >= 0:
                        a = TCH * jj + los[jj]
                        nc.vector.tensor_mul(
                            ext[:, a : a + 128], ext[:, a : a + 128], mask_t[:]
                        )
                return ext, los

            def emit_pv(tch, jp, acc, ext, los):
                jmax = 4 * tch + 3
                for jj in range(2):
                    j = jp + jj
                    lo = los[jj]
                    nc.tensor.matmul(
                        acc[:, lo:TCH] if j > 0 else acc[:, :],
                        v1[j][:],
                        ext[:, TCH * jj + lo : TCH * (jj + 1)],
                        start=(j == 0),
                        stop=(j == jmax),
                        skip_group_check=True,
                    )

            def emit_epilogue(tch, acc):
                # ======== normalize + transpose + DMA out for chunk tch ====
                oT = epip.tile([65, TCH], f32, tag="oT", name=f"oT{tch}")
                nc.vector.tensor_copy(oT[:], acc[:])
                ot = epip.tile([128, 4, H], f32, tag="ot", name=f"ot{tch}")
                for i in range(4):
                    Pe = miscp.tile([128, 65], f32, tag="misc", name=f"Pe{tch}_{i}")
                    nc.tensor.transpose(
                        Pe[:, :],
                        oT[:, 128 * i : 128 * (i + 1)],
                        idf_t[0:65, 0:65],
                    )
                    rec = epip.tile([128, 1], f32, tag="rec", name=f"rec{tch}_{i}")
                    nc.vector.reciprocal(rec[:], Pe[:, 64:65])
                    nc.vector.tensor_scalar_mul(ot[:, i, :], Pe[:, 0:64], rec[:])
                r0 = TCH * tch
                nc.sync.dma_start(
                    out=out_d[r0 : r0 + TCH, :].rearrange("(i p) h -> p i h", i=4),
                    in_=ot[:],
                )

            # ---- chunk 0: qk proj, then scores+exp of pair 0 immediately
            # (unpacked, no q/k duplication round-trip on the critical path),
            # with the v projection filling the PE while the exp runs. ----
            qk0, v0 = proj_steps(0)
            for s in qk0:
                s()
            acc = accp.tile([65, TCH], f32, tag="acc", name="acc0")
            ext0, los0 = emit_scores_exp(0, 0, unpacked=True)
            for s in v0:
                s()
            ext1, los1 = emit_scores_exp(0, 2, unpacked=True)
            emit_pv(0, 0, acc, ext0, los0)
            # chunk-1 qk projection (+ q/k copies) rides between chunk-0 PVs
            qk_next, v_own = proj_steps(1)
            for s in qk_next:
                s()
            emit_pv(0, 2, acc, ext1, los1)
            prev_epi = (0, acc)

            for tch in range(1, N_CHUNK):
                # At this point chunk tch's qk projection is fully emitted
                # and v_own holds chunk tch's v-projection steps (13 items:
                # needed only by this chunk's diagonal pairs). qk_next of
                # chunk tch+1 (9 items) is paced across the pairs so its q/k
                # duplication DMAs have a full window to land. Scores run one
                # pair ahead of PV so the exp stream never waits on paced
                # projection work.
                if tch + 1 < N_CHUNK:
                    qk_next, v_next = proj_steps(tch + 1)
                else:
                    qk_next, v_next = [], []
                jmax = 4 * tch + 3
                pairs = list(range(0, jmax + 1, 2))
                n_pairs = len(pairs)
                first_diag = 2 * tch  # pair index of the first diagonal pair
                unpacked = tch <= 1
                acc = accp.tile([65, TCH], f32, tag="acc", name=f"acc{tch}")
                prev = None
                for pi, jp in enumerate(pairs):
                    ext, los = emit_scores_exp(tch, jp, unpacked=unpacked)
                    if prev is not None:
                        if prev[0] >= 4 * tch:  # diagonal pair: needs own v1
                            while v_own:
                                v_own.pop(0)()
                        emit_pv(tch, prev[0], acc, prev[1], prev[2])
                    if pi == 0 and prev_epi is not None:
                        # previous chunk's epilogue rides behind this chunk's
                        # first scores so the PE never stalls on the acc copy
                        emit_epilogue(*prev_epi)
                    # pace interleaved work: own v before the first diagonal
                    # pair, next chunk's qk spread over all pairs
                    if v_own:
                        slots = max(1, first_diag + 1 - pi)
                        for _ in range(-(-len(v_own) // slots)):
                            if v_own:
                                v_own.pop(0)()
                    if qk_next:
                        slots = max(1, n_pairs - pi)
                        for _ in range(-(-len(qk_next) // slots)):
                            if qk_next:
                                qk_next.pop(0)()
                    prev = (jp, ext, los)
                if prev[0] >= 4 * tch:
                    while v_own:
                        v_own.pop(0)()
                emit_pv(tch, prev[0], acc, prev[1], prev[2])
                for s in qk_next:
                    s()
                v_own = v_next
                prev_epi = (tch, acc)

            emit_epilogue(*prev_epi)

    nc.compile()
    return nc


def _get_nc():
    if "nc" not in _CACHE:
        _CACHE["nc"] = _build()
    return _CACHE["nc"]


def _tile_w(w):
    """[C, F] -> [128, N_CT*F] with c-tile-major column blocks."""
    Cdim, F = w.shape
    return np.ascontiguousarray(
        w.reshape(Cdim // 128, 128, F).transpose(1, 0, 2).reshape(128, -1)
    )


def _host_inputs(x, w_q, w_k, w_v):
    bf = ml_dtypes.bfloat16
    x = np.asarray(x, dtype=np.float32)
    wqk = np.concatenate(
        [np.asarray(w_q, np.float32), np.asarray(w_k, np.float32)], 1
    )
    wv = np.asarray(w_v, np.float32)
    wqk_tiled = _tile_w(wqk).astype(bf)
    wv_tiled = _tile_w(wv).astype(bf)
    # multiplicative causal mask for transposed-score diag blocks: keep s <= t
    mask01 = np.triu(np.ones((128, 128), np.float32)).astype(bf)
    idf = np.eye(128, dtype=np.float32)
    idb = np.eye(128, dtype=np.float32).astype(bf)
    in_maps = []
    for i in range(N_CORES):
        # x^T pre-tiled: [128, chunk, c-tile, t] flattened per partition
        xT = np.ascontiguousarray(x[i].T).astype(bf)  # [C, T]
        xT4 = xT.reshape(N_CT, 128, N_CHUNK, TCH)     # [c, p, chunk, t]
        xTt = np.ascontiguousarray(
            xT4.transpose(1, 2, 0, 3).reshape(128, -1)
        )
        in_maps.append(
            {
                "xT": xTt,
                "wqk": wqk_tiled,
                "wv": wv_tiled,
                "mask01": mask01,
                "idf": idf,
                "idb": idb,
            }
        )
    return in_maps


def run(x, w_q, w_k, w_v, trace=False, **trace_kwargs):
    from concourse.bass_utils import run_bass_kernel_spmd

    nc = _get_nc()
    in_maps = _host_inputs(x, w_q, w_k, w_v)
    res = run_bass_kernel_spmd(
        nc, in_maps, core_ids=list(range(N_CORES)), trace=trace, **trace_kwargs
    )
    out = np.stack([np.asarray(res.results[i]["out"]) for i in range(N_CORES)])
    return out.astype(np.float32), res


def kernel(x, w_q, w_k, w_v):
    out, _ = run(x, w_q, w_k, w_v, trace=False)
    return out


# revision 16
# speedup vs baseline: 1.1208x; 1.1208x over previous
"""Distributed Trainium2 kernel for a single attention head.

Problem: x:[8,2048,1024] f32, w_q/w_k/w_v:[1024,64] f32
  q,k,v = x@w ; scores = (q k^T)/sqrt(1024) causal-masked; out = softmax(scores)@v

Sharding: data-parallel over batch B=8 across the 8 NeuronCores (one batch
element per core, weights replicated, no collectives).

Per-core dataflow (T=2048, C=1024, H=64):
  - host ships x^T pre-tiled [128, chunk, c, 512] in bf16, packed w_qk / w_v
    (bf16), a 0/1 triangular mask tile, and identities for transposes.
  - ~10 warm-up matmuls on a scratch tile run while the input DMA streams so
    the PE HAM clock-gate reaches 8/8 before real work arrives.
  - all x/weight input DMAs ride the sync HWDGE queue (large transfers,
    chunk-0 split fine for early start); q/k partition-duplication and output
    DMAs ride the gpsimd SWDGE queue so they never queue behind the x stream.
  - projections with weights stationary (bf16): qkT [128,T] (q rows 0:64,
    k rows 64:128), vT [64,T]; q/k copied to SBUF and duplicated onto both
    partition halves so score pairs can run 2x row-packed on the PE.
  - scores computed TRANSPOSED per s-tile: S[s,t] = kT_slice.T @ qT (K=64),
    two s-tiles concurrently in PE row-groups 0/1; diagonal s-tiles only
    compute the columns t >= 128*rel that survive the causal mask.
  - exp on ScalarE with scale=1/32 folded in (|scores|<~2, no max needed),
    one [128,1024] activation per pair, output bf16; the ACT table set is
    pre-loaded by a dummy exp at t~0.
  - causal: diagonal 128x128 blocks multiplied by a 0/1 lower-triangle mask
    on VectorE after the exp (keeps the PE free of mask matmuls).
  - PV: out^T[h,t] accumulated over s-tiles with lhsT = [v | 1] so row 64 of
    the accumulator is the softmax denominator (fused row-sum).
  - epilogue: TensorE transpose back to [t,h], reciprocal-multiply on
    VectorE, one combined [512,64] DMA out per chunk.
  - next-chunk projections are emission-interleaved between attention pairs
    so the PE stream stays dense.
"""

import os
import sys

import numpy as np

for p in ("/opt/trn_rl_repo",):
    if p not in sys.path and os.path.isdir(p):
        sys.path.insert(0, p)

import ml_dtypes  # noqa: E402

B, T, C, H = 8, 2048, 1024, 64
N_CORES = 8
TCH = 512                  # t-chunk (columns per PSUM bank of f32)
N_CHUNK = T // TCH         # 4
N_CT = C // 128            # 8 contraction tiles
SCALE = float(C) ** -0.5   # 1/32
N_WARM = 10                # PE warm-up matmuls

_CACHE = {}


def _build():
    """Build + compile the SPMD Bass graph (same graph on all 8 cores)."""
    import concourse.bass as bass
    import concourse.mybir as mybir
    import concourse.tile as tile
    from concourse import bacc

    f32 = mybir.dt.float32
    bf16 = mybir.dt.bfloat16
    EXP = mybir.ActivationFunctionType.Exp

    nc = bacc.Bacc(
        "TRN2", target_bir_lowering=False, debug=False, num_devices=N_CORES
    )

    # host ships one "blob0" = [wv | wqk | x^T chunk 0] so the entire
    # critical-path input is a single max-rate DMA, plus the remaining x^T
    # chunks pre-tiled [c-tile][t] per partition (one contiguous DMA each).
    BLOB0_W = N_CT * H + N_CT * 128 + N_CT * TCH  # 512 + 1024 + 4096
    blob0_d = nc.dram_tensor("blob0", [128, BLOB0_W], bf16, kind="ExternalInput")
    xrest_d = nc.dram_tensor(
        "xrest", [128, (N_CHUNK - 1) * N_CT * TCH], bf16, kind="ExternalInput"
    )
    mask_d = nc.dram_tensor("mask01", [128, 128], bf16, kind="ExternalInput")  # 0/1, keep s<=t
    idf_d = nc.dram_tensor("idf", [128, 128], f32, kind="ExternalInput")
    idb_d = nc.dram_tensor("idb", [128, 128], bf16, kind="ExternalInput")
    out_d = nc.dram_tensor("out", [T, H], f32, kind="ExternalOutput")

    with tile.TileContext(nc) as tc:
        with (
            tc.tile_pool(name="const", bufs=1) as constp,
            tc.tile_pool(name="xTp", bufs=1) as xTp,
            tc.tile_pool(name="qkp", bufs=1) as qkp,
            tc.tile_pool(name="q2p", bufs=2) as q2p,
            tc.tile_pool(name="vTp", bufs=2) as vTp,
            tc.tile_pool(name="v1p", bufs=1) as v1p,
            tc.tile_pool(name="exp", bufs=4) as expp,
            tc.tile_pool(name="epi", bufs=2) as epip,
            tc.tile_pool(name="Sp", bufs=2, space="PSUM") as Sp,
            tc.tile_pool(name="accp", bufs=1, space="PSUM") as accp,
            tc.tile_pool(name="miscp", bufs=3, space="PSUM") as miscp,
        ):
            # ---- PE warm-up scratch ----
            warm_sb = constp.tile([128, 256], bf16, tag="warm_sb", name="warm_sb")
            nc.vector.memset(warm_sb[:], 0.0)
            warm_act = constp.tile([128, 8], bf16, tag="warm_act", name="warm_act")
            warm_ps = miscp.tile([128, 256], f32, tag="misc", name="warm_ps")
            for i in range(N_WARM):
                nc.tensor.matmul(
                    warm_ps[:, :],
                    warm_sb[:, 0:128],
                    warm_sb[:, :],
                    start=True,
                    stop=True,
                    skip_group_check=True,
                )

            # ---- input DMAs: all on the sync HWDGE queue, strict FIFO, so
            # chunk-0's blob (wv+wqk+x0) gets every SDMA engine first and the
            # rest streams behind it in deadline order. ----
            blob0_t = constp.tile([128, BLOB0_W], bf16, tag="blob0", name="blob0_t")
            nc.sync.dma_start(out=blob0_t[:], in_=blob0_d[:])
            wv_t = blob0_t[:, 0 : N_CT * H].rearrange("p (n m) -> p n m", n=N_CT)
            wqk_t = blob0_t[:, N_CT * H : N_CT * (H + 128)].rearrange(
                "p (n m) -> p n m", n=N_CT
            )
            xt = {
                0: blob0_t[:, N_CT * (H + 128) : BLOB0_W].rearrange(
                    "p (n m) -> p n m", n=N_CT
                )
            }
            xr_v = xrest_d[:].rearrange(
                "p (t n m) -> p t n m", t=N_CHUNK - 1, n=N_CT
            )
            for t in range(1, N_CHUNK):
                xx = xTp.tile([128, N_CT, TCH], bf16, tag=f"x{t}", name=f"x{t}")
                nc.sync.dma_start(out=xx[:], in_=xr_v[:, t - 1, :, :])
                xt[t] = xx[:]

            # dummy exp: forces the ACT table-set load to overlap the
            # input-DMA phase (scalar issues no DMAs in this schedule)
            nc.scalar.activation(warm_act[:], warm_sb[:, 0:8], EXP, scale=1.0)

            # ---- small constants on the gpsimd SWDGE queue ----
            mask_t = constp.tile([128, 128], bf16, tag="mask", name="mask_t")
            nc.gpsimd.dma_start(out=mask_t[:], in_=mask_d[:])
            idb_t = constp.tile([128, 128], bf16, tag="idb", name="idb_t")
            nc.gpsimd.dma_start(out=idb_t[:], in_=idb_d[:])
            idf_t = constp.tile([128, 128], f32, tag="idf", name="idf_t")
            nc.gpsimd.dma_start(out=idf_t[:], in_=idf_d[:])

            qk2 = {}   # [128, TCH] bf16: qT duplicated on both partition halves
            kk2 = {}   # [128, TCH] bf16: kT duplicated on both partition halves
            v1 = {}    # [128, 65] bf16 per s-tile: [v | 1]

            def proj_steps(tch):
                """Emission thunks for chunk `tch`: (qk steps, v steps)."""
                qk_steps = []
                v_steps = []
                state = {}

                def qk_mm(c):
                    def f():
                        if c == 0:
                            state["S"] = miscp.tile(
                                [128, TCH], f32, tag="misc", name=f"Sqk{tch}"
                            )
                        nc.tensor.matmul(
                            state["S"][:, :],
                            wqk_t[:, c, :],
                            xt[tch][:, c, :],
                            start=(c == 0),
                            stop=(c == N_CT - 1),
                            skip_group_check=True,
                        )
                    return f

                def qk_out():
                    S = state["S"]
                    q2 = q2p.tile([128, TCH], bf16, tag="q2", name=f"q2_{tch}")
                    k2 = qkp.tile([128, TCH], bf16, tag=f"k2_{tch}", name=f"k2_{tch}")
                    nc.vector.tensor_copy(k2[0:64, :], S[64:128, :])
                    nc.vector.tensor_copy(q2[0:64, :], S[0:64, :])
                    # chunks 0-1 run their scores unpacked on partition half
                    # 0 (the ~2.5us SWDGE duplication round-trip would sit on
                    # their critical path); their q never needs duplicating.
                    # k still does (used by chunks 2-3 in row-packed pairs).
                    if tch > 1:
                        nc.gpsimd.dma_start(out=q2[64:128, :], in_=q2[0:64, :])
                    nc.gpsimd.dma_start(out=k2[64:128, :], in_=k2[0:64, :])
                    qk2[tch] = q2
                    kk2[tch] = k2

                def v_mm(c):
                    def f():
                        if c == 0:
                            state["Pv"] = miscp.tile(
                                [64, TCH], f32, tag="misc", name=f"Pv{tch}"
                            )
                        nc.tensor.matmul(
                            state["Pv"][:, :],
                            wv_t[:, c, :],
                            xt[tch][:, c, :],
                            start=(c == 0),
                            stop=(c == N_CT - 1),
                            skip_group_check=True,
                        )
                    return f

                def v_out():
                    vTt = vTp.tile([64, TCH], bf16, tag="vT", name=f"vT{tch}")
                    nc.vector.tensor_copy(vTt[:], state["Pv"][:, :])
                    state["vT"] = vTt

                def v1_build(i):
                    def f():
                        j = 4 * tch + i
                        Pt = miscp.tile([128, H], bf16, tag="misc", name=f"Pt{j}")
                        nc.tensor.transpose(
                            Pt[:, :],
                            state["vT"][:, 128 * i : 128 * (i + 1)],
                            idb_t[0:64, 0:64],
                        )
                        v1t = v1p.tile([128, 65], bf16, tag=f"v1_{j}", name=f"v1_{j}")
                        nc.vector.tensor_copy(v1t[:, 0:64], Pt[:, :])
                        nc.vector.memset(v1t[:, 64:65], 1.0)
                        v1[j] = v1t
                    return f

                for c in range(N_CT):
                    qk_steps.append(qk_mm(c))
                qk_steps.append(qk_out)
                for c in range(N_CT):
                    v_steps.append(v_mm(c))
                v_steps.append(v_out)
                for i in range(4):
                    v_steps.append(v1_build(i))
                return qk_steps, v_steps

            def emit_scores_exp(tch, jp, unpacked):
                """Scores matmuls + exp for pair (jp, jp+1); returns (ext, los)."""
                S2 = Sp.tile([128, 2 * TCH], f32, tag="S", name=f"S{tch}_{jp}")
                los = {}
                for jj in range(2):
                    j = jp + jj
                    rel = j - 4 * tch
                    lo = 128 * max(0, rel)
                    los[jj] = lo
                    half = slice(0, 64) if unpacked else slice(64 * jj, 64 * (jj + 1))
                    ksl = kk2[j // 4][half, 128 * (j % 4) : 128 * (j % 4 + 1)]
                    nc.tensor.matmul(
                        S2[:, TCH * jj + lo : TCH * (jj + 1)],
                        ksl,
                        qk2[tch][half, lo:TCH],
                        start=True,
                        stop=True,
                        skip_group_check=True,
                    )
                ext = expp.tile([128, 2 * TCH], bf16, tag="ex", name=f"ex{tch}_{jp}")
                nc.scalar.activation(ext[:], S2[:], EXP, scale=SCALE)
                # causal 0/1 mask on the diagonal 128x128 blocks (VectorE)
                for jj in range(2):
                    j = jp + jj
                    if j - 4 * tch >= 0:
                        a = TCH * jj + los[jj]
                        nc.vector.tensor_mul(
                            ext[:, a : a + 128], ext[:, a : a + 128], mask_t[:]
                        )
                return ext, los

            def emit_pv(tch, jp, acc, ext, los):
                jmax = 4 * tch + 3
                for jj in range(2):
                    j = jp + jj
                    lo = los[jj]
                    nc.tensor.matmul(
                        acc[:, lo:TCH] if j > 0 else acc[:, :],
                        v1[j][:],
                        ext[:, TCH * jj + lo : TCH * (jj + 1)],
                        start=(j == 0),
                        stop=(j == jmax),
                        skip_group_check=True,
                    )

            def emit_epilogue(tch, acc):
                # ======== normalize + transpose + DMA out for chunk tch ====
                oT = epip.tile([65, TCH], f32, tag="oT", name=f"oT{tch}")
                nc.vector.tensor_copy(oT[:], acc[:])
                ot = epip.tile([128, 4, H], f32, tag="ot", name=f"ot{tch}")
                for i in range(4):
                    Pe = miscp.tile([128, 65], f32, tag="misc", name=f"Pe{tch}_{i}")
                    nc.tensor.transpose(
                        Pe[:, :],
                        oT[:, 128 * i : 128 * (i + 1)],
                        idf_t[0:65, 0:65],
                    )
                    rec = epip.tile([128, 1], f32, tag="rec", name=f"rec{tch}_{i}")
                    nc.vector.reciprocal(rec[:], Pe[:, 64:65])
                    nc.vector.tensor_scalar_mul(ot[:, i, :], Pe[:, 0:64], rec[:])
                r0 = TCH * tch
                nc.sync.dma_start(
                    out=out_d[r0 : r0 + TCH, :].rearrange("(i p) h -> p i h", i=4),
                    in_=ot[:],
                )

            # ---- chunk 0: qk proj, then scores+exp of pair 0 immediately
            # (unpacked, no q/k duplication round-trip on the critical path),
            # with the v projection filling the PE while the exp runs. ----
            qk0, v0 = proj_steps(0)
            for s in qk0:
                s()
            acc = accp.tile([65, TCH], f32, tag="acc", name="acc0")
            ext0, los0 = emit_scores_exp(0, 0, unpacked=True)
            for s in v0:
                s()
            ext1, los1 = emit_scores_exp(0, 2, unpacked=True)
            emit_pv(0, 0, acc, ext0, los0)
            # chunk-1 qk projection (+ q/k copies) rides between chunk-0 PVs
            qk_next, v_own = proj_steps(1)
            for s in qk_next:
                s()
            emit_pv(0, 2, acc, ext1, los1)
            prev_epi = (0, acc)

            for tch in range(1, N_CHUNK):
                # At this point chunk tch's qk projection is fully emitted
                # and v_own holds chunk tch's v-projection steps (13 items:
                # needed only by this chunk's diagonal pairs). qk_next of
                # chunk tch+1 (9 items) is paced across the pairs so its q/k
                # duplication DMAs have a full window to land. Scores run one
                # pair ahead of PV so the exp stream never waits on paced
                # projection work.
                if tch + 1 < N_CHUNK:
                    qk_next, v_next = proj_steps(tch + 1)
                else:
                    qk_next, v_next = [], []
                jmax = 4 * tch + 3
                pairs = list(range(0, jmax + 1, 2))
                n_pairs = len(pairs)
                first_diag = 2 * tch  # pair index of the first diagonal pair
                unpacked = tch <= 1
                acc = accp.tile([65, TCH], f32, tag="acc", name=f"acc{tch}")
                prev = None
                for pi, jp in enumerate(pairs):
                    ext, los = emit_scores_exp(tch, jp, unpacked=unpacked)
                    if prev is not None:
                        if prev[0] >= 4 * tch:  # diagonal pair: needs own v1
                            while v_own:
                                v_own.pop(0)()
                        emit_pv(tch, prev[0], acc, prev[1], prev[2])
                    if pi == 0 and prev_epi is not None:
                        # previous chunk's epilogue rides behind this chunk's
                        # first scores so the PE never stalls on the acc copy
                        emit_epilogue(*prev_epi)
                    # pace interleaved work: own v before the first diagonal
                    # pair, next chunk's qk spread over all pairs
                    if v_own:
                        slots = max(1, first_diag + 1 - pi)
                        for _ in range(-(-len(v_own) // slots)):
                            if v_own:
                                v_own.pop(0)()
                    if qk_next:
                        slots = max(1, n_pairs - pi)
                        for _ in range(-(-len(qk_next) // slots)):
                            if qk_next:
                                qk_next.pop(0)()
                    prev = (jp, ext, los)
                if prev[0] >= 4 * tch:
                    while v_own:
                        v_own.pop(0)()
                emit_pv(tch, prev[0], acc, prev[1], prev[2])
                for s in qk_next:
                    s()
                v_own = v_next
                prev_epi = (tch, acc)

            emit_epilogue(*prev_epi)

    nc.compile()
    return nc


def _get_nc():
    if "nc" not in _CACHE:
        _CACHE["nc"] = _build()
    return _CACHE["nc"]


def _tile_w(w):
    """[C, F] -> [128, N_CT*F] with c-tile-major column blocks."""
    Cdim, F = w.shape
    return np.ascontiguousarray(
        w.reshape(Cdim // 128, 128, F).transpose(1, 0, 2).reshape(128, -1)
    )


def _host_inputs(x, w_q, w_k, w_v):
    bf = ml_dtypes.bfloat16
    x = np.asarray(x, dtype=np.float32)
    wqk = np.concatenate(
        [np.asarray(w_q, np.float32), np.asarray(w_k, np.float32)], 1
    )
    wv = np.asarray(w_v, np.float32)
    wqk_tiled = _tile_w(wqk).astype(bf)
    wv_tiled = _tile_w(wv).astype(bf)
    # multiplicative causal mask for transposed-score diag blocks: keep s <= t
    mask01 = np.triu(np.ones((128, 128), np.float32)).astype(bf)
    idf = np.eye(128, dtype=np.float32)
    idb = np.eye(128, dtype=np.float32).astype(bf)
    in_maps = []
    for i in range(N_CORES):
        # x^T pre-tiled: [128, chunk, c-tile, t] flattened per partition
        xT = np.ascontiguousarray(x[i].T).astype(bf)  # [C, T]
        xT4 = xT.reshape(N_CT, 128, N_CHUNK, TCH)     # [c, p, chunk, t]
        xTt = xT4.transpose(1, 2, 0, 3).reshape(128, N_CHUNK, -1)  # [p, chunk, c*t]
        blob0 = np.ascontiguousarray(
            np.concatenate([wv_tiled, wqk_tiled, xTt[:, 0, :]], axis=1)
        )
        xrest = np.ascontiguousarray(xTt[:, 1:, :].reshape(128, -1))
        in_maps.append(
            {
                "blob0": blob0,
                "xrest": xrest,
                "mask01": mask01,
                "idf": idf,
                "idb": idb,
            }
        )
    return in_maps


def run(x, w_q, w_k, w_v, trace=False, **trace_kwargs):
    from concourse.bass_utils import run_bass_kernel_spmd

    nc = _get_nc()
    in_maps = _host_inputs(x, w_q, w_k, w_v)
    res = run_bass_kernel_spmd(
        nc, in_maps, core_ids=list(range(N_CORES)), trace=trace, **trace_kwargs
    )
    out = np.stack([np.asarray(res.results[i]["out"]) for i in range(N_CORES)])
    return out.astype(np.float32), res


def kernel(x, w_q, w_k, w_v):
    out, _ = run(x, w_q, w_k, w_v, trace=False)
    return out


# revision 19
# speedup vs baseline: 1.1473x; 1.0236x over previous
"""Distributed Trainium2 kernel for a single attention head.

Problem: x:[8,2048,1024] f32, w_q/w_k/w_v:[1024,64] f32
  q,k,v = x@w ; scores = (q k^T)/sqrt(1024) causal-masked; out = softmax(scores)@v

Sharding: data-parallel over batch B=8 across the 8 NeuronCores (one batch
element per core, weights replicated, no collectives).

Per-core dataflow (T=2048, C=1024, H=64), built around keeping the ScalarE
exp stream (the serial resource: 20 x ~1.1us activations) gap-free:

  - host ships blob0 = [wv | wqk | x^T chunks 0-1] bf16 as ONE max-rate DMA
    at the head of the sync HWDGE queue, then x^T chunks 2-3 (one DMA each).
    q/k partition-duplication + output DMAs ride the gpsimd SWDGE queue.
  - 16 N=512 warm-up matmuls bridge the PE from the preamble to the blob0
    arrival so the HAM clock-gate is at 8/8 when real work starts.
  - projections with weights stationary (bf16): qkT [128,T] (q rows 0:64,
    k rows 64:128) and vT [64,T] per chunk.
  - scores computed TRANSPOSED per s-tile pair: S[s,t] = kT_slice.T @ qT
    (K=64), written to f32 PSUM tiles [128,1024] (two banks each, Sp
    bufs=2), so two score pairs can be banked ahead of the exp stream.
  - chunks 0-1 run scores unpacked on partition half 0 (no dup round-trip on
    the critical path); chunks 2-3 row-packed 2x in PE row-groups 0/1.
  - exp on ScalarE, scale=1/32 folded in (|scores|<~2, no max needed), one
    [128,1024] activation per pair -> bf16 SBUF; table set pre-loaded by a
    dummy exp during the DMA phase.
  - causal: diagonal 128x128 blocks multiplied by a 0/1 mask on VectorE
    after the exp (keeps the PE free of mask matmuls).
  - PV: out^T[h,t] accumulated per chunk with lhsT = [v | 1] so row 64 is
    the softmax denominator (fused row-sum). PV for pair k is emitted LAG=4
    slots behind its scores, so the in-order PE stream never blocks scores
    production on exp completion.
  - projection work for later chunks is paced between pair slots with
    explicit due-dates (qk of chunk c before its first scores; v of chunk c
    before its first diagonal PV).
  - epilogue: TensorE transpose back to [t,h], reciprocal-multiply on
    VectorE, one combined [512,64] DMA out per chunk.
"""

import os
import sys

import numpy as np

for p in ("/opt/trn_rl_repo",):
    if p not in sys.path and os.path.isdir(p):
        sys.path.insert(0, p)

import ml_dtypes  # noqa: E402

B, T, C, H = 8, 2048, 1024, 64
N_CORES = 8
TCH = 512                  # t-chunk
N_CHUNK = T // TCH         # 4
N_CT = C // 128            # 8 contraction tiles
SCALE = float(C) ** -0.5   # 1/32
N_WARM = 16                # PE warm-up matmuls (N=512 each, ~0.43us cold)
LAG = 4                    # PV trails scores by this many pair slots

_CACHE = {}


def _build():
    """Build + compile the SPMD Bass graph (same graph on all 8 cores)."""
    import concourse.bass as bass
    import concourse.mybir as mybir
    import concourse.tile as tile
    from concourse import bacc

    f32 = mybir.dt.float32
    bf16 = mybir.dt.bfloat16
    EXP = mybir.ActivationFunctionType.Exp

    nc = bacc.Bacc(
        "TRN2", target_bir_lowering=False, debug=False, num_devices=N_CORES
    )

    # blob0 = [wv | wqk | x^T chunk0 | x^T chunk1], x pre-tiled [c-tile][t]
    W_WV = N_CT * H
    W_WQK = N_CT * 128
    W_X = N_CT * TCH
    BLOB0_W = W_WV + W_WQK + 2 * W_X
    blob0_d = nc.dram_tensor("blob0", [128, BLOB0_W], bf16, kind="ExternalInput")
    xrest_d = nc.dram_tensor("xrest", [128, 2 * W_X], bf16, kind="ExternalInput")
    mask_d = nc.dram_tensor("mask01", [128, 128], bf16, kind="ExternalInput")
    idf_d = nc.dram_tensor("idf", [128, 128], f32, kind="ExternalInput")
    idb_d = nc.dram_tensor("idb", [128, 128], bf16, kind="ExternalInput")
    out_d = nc.dram_tensor("out", [T, H], f32, kind="ExternalOutput")

    with tile.TileContext(nc) as tc:
        with (
            tc.tile_pool(name="const", bufs=1) as constp,
            tc.tile_pool(name="xTp", bufs=1) as xTp,
            tc.tile_pool(name="qkp", bufs=1) as qkp,
            tc.tile_pool(name="q2p", bufs=2) as q2p,
            tc.tile_pool(name="vTp", bufs=2) as vTp,
            tc.tile_pool(name="v1p", bufs=1) as v1p,
            tc.tile_pool(name="exp", bufs=LAG + 2) as expp,
            tc.tile_pool(name="epi", bufs=2) as epip,
            tc.tile_pool(name="Sp", bufs=2, space="PSUM") as Sp,
            tc.tile_pool(name="accp", bufs=1, space="PSUM") as accp,
            tc.tile_pool(name="miscp", bufs=3, space="PSUM") as miscp,
        ):
            # ---- PE warm-up scratch ----
            warm_sb = constp.tile([128, TCH], bf16, tag="warm_sb", name="warm_sb")
            nc.vector.memset(warm_sb[:], 0.0)
            warm_act = constp.tile([128, 8], bf16, tag="warm_act", name="warm_act")
            warm_ps = miscp.tile([128, TCH], f32, tag="misc", name="warm_ps")
            for i in range(N_WARM):
                nc.tensor.matmul(
                    warm_ps[:, :],
                    warm_sb[:, 0:128],
                    warm_sb[:, :],
                    start=True,
                    stop=True,
                    skip_group_check=True,
                )

            # ---- input DMAs: all on the sync HWDGE queue, strict FIFO ----
            blob0_t = constp.tile([128, BLOB0_W], bf16, tag="blob0", name="blob0_t")
            nc.sync.dma_start(out=blob0_t[:], in_=blob0_d[:])
            wv_t = blob0_t[:, 0:W_WV].rearrange("p (n m) -> p n m", n=N_CT)
            wqk_t = blob0_t[:, W_WV : W_WV + W_WQK].rearrange(
                "p (n m) -> p n m", n=N_CT
            )
            xt = {}
            for t in range(2):
                a = W_WV + W_WQK + t * W_X
                xt[t] = blob0_t[:, a : a + W_X].rearrange("p (n m) -> p n m", n=N_CT)
            xr_v = xrest_d[:].rearrange("p (t n m) -> p t n m", t=2, n=N_CT)
            for t in (2, 3):
                xx = xTp.tile([128, N_CT, TCH], bf16, tag=f"x{t}", name=f"x{t}")
                nc.sync.dma_start(out=xx[:], in_=xr_v[:, t - 2, :, :])
                xt[t] = xx[:]

            # dummy exp: forces the ACT table-set load during the DMA phase
            nc.scalar.activation(warm_act[:], warm_sb[:, 0:8], EXP, scale=1.0)

            # ---- small constants on the gpsimd SWDGE queue ----
            mask_t = constp.tile([128, 128], bf16, tag="mask", name="mask_t")
            nc.gpsimd.dma_start(out=mask_t[:], in_=mask_d[:])
            idb_t = constp.tile([128, 128], bf16, tag="idb", name="idb_t")
            nc.gpsimd.dma_start(out=idb_t[:], in_=idb_d[:])
            idf_t = constp.tile([128, 128], f32, tag="idf", name="idf_t")
            nc.gpsimd.dma_start(out=idf_t[:], in_=idf_d[:])

            qk2 = {}   # [128, TCH] bf16 per chunk: qT (dup'd for chunks 2-3)
            kk2 = {}   # [128, TCH] bf16 per chunk: kT dup'd on both halves
            v1 = {}    # [128, 65] bf16 per s-tile: [v | 1]

            def proj_steps(tch):
                """Emission thunks for chunk `tch`: (qk steps, v steps)."""
                qk_steps = []
                v_steps = []
                state = {}

                def qk_mm(c):
                    def f():
                        if c == 0:
                            state["S"] = miscp.tile(
                                [128, TCH], f32, tag="misc", name=f"Sqk{tch}"
                            )
                        nc.tensor.matmul(
                            state["S"][:, :],
                            wqk_t[:, c, :],
                            xt[tch][:, c, :],
                            start=(c == 0),
                            stop=(c == N_CT - 1),
                            skip_group_check=True,
                        )
                    return f

                def qk_out():
                    S = state["S"]
                    q2 = q2p.tile([128, TCH], bf16, tag="q2", name=f"q2_{tch}")
                    k2 = qkp.tile([128, TCH], bf16, tag=f"k2_{tch}", name=f"k2_{tch}")
                    nc.vector.tensor_copy(k2[0:64, :], S[64:128, :])
                    nc.vector.tensor_copy(q2[0:64, :], S[0:64, :])
                    # chunks 0-1 score unpacked on half 0: q never needs the
                    # SWDGE dup round-trip; k still does (chunks 2-3 pack).
                    if tch > 1:
                        nc.gpsimd.dma_start(out=q2[64:128, :], in_=q2[0:64, :])
                    nc.gpsimd.dma_start(out=k2[64:128, :], in_=k2[0:64, :])
                    qk2[tch] = q2
                    kk2[tch] = k2

                def v_mm(c):
                    def f():
                        if c == 0:
                            state["Pv"] = miscp.tile(
                                [64, TCH], f32, tag="misc", name=f"Pv{tch}"
                            )
                        nc.tensor.matmul(
                            state["Pv"][:, :],
                            wv_t[:, c, :],
                            xt[tch][:, c, :],
                            start=(c == 0),
                            stop=(c == N_CT - 1),
                            skip_group_check=True,
                        )
                    return f

                def v_out():
                    vTt = vTp.tile([64, TCH], bf16, tag="vT", name=f"vT{tch}")
                    nc.vector.tensor_copy(vTt[:], state["Pv"][:, :])
                    state["vT"] = vTt

                def v1_build(i):
                    def f():
                        j = 4 * tch + i
                        Pt = miscp.tile([128, H], bf16, tag="misc", name=f"Pt{j}")
                        nc.tensor.transpose(
                            Pt[:, :],
                            state["vT"][:, 128 * i : 128 * (i + 1)],
                            idb_t[0:64, 0:64],
                        )
                        v1t = v1p.tile([128, 65], bf16, tag=f"v1_{j}", name=f"v1_{j}")
                        nc.vector.tensor_copy(v1t[:, 0:64], Pt[:, :])
                        nc.vector.memset(v1t[:, 64:65], 1.0)
                        v1[j] = v1t
                    return f

                for c in range(N_CT):
                    qk_steps.append(qk_mm(c))
                qk_steps.append(qk_out)
                for c in range(N_CT):
                    v_steps.append(v_mm(c))
                v_steps.append(v_out)
                for i in range(4):
                    v_steps.append(v1_build(i))
                return qk_steps, v_steps

            def emit_scores_exp(tch, jp):
                """Scores matmuls + exp + masks for pair (jp, jp+1)."""
                unpacked = tch <= 1
                S2 = Sp.tile([128, 2 * TCH], f32, tag="S", name=f"S{tch}_{jp}")
                los = {}
                for jj in range(2):
                    j = jp + jj
                    rel = j - 4 * tch
                    lo = 128 * max(0, rel)
                    los[jj] = lo
                    half = slice(0, 64) if unpacked else slice(64 * jj, 64 * (jj + 1))
                    ksl = kk2[j // 4][half, 128 * (j % 4) : 128 * (j % 4 + 1)]
                    nc.tensor.matmul(
                        S2[:, TCH * jj + lo : TCH * (jj + 1)],
                        ksl,
                        qk2[tch][half, lo:TCH],
                        start=True,
                        stop=True,
                        skip_group_check=True,
                    )
                ext = expp.tile([128, 2 * TCH], bf16, tag="ex", name=f"ex{tch}_{jp}")
                nc.scalar.activation(ext[:], S2[:], EXP, scale=SCALE)
                # causal 0/1 mask on the diagonal 128x128 blocks (VectorE)
                for jj in range(2):
                    if jp + jj - 4 * tch >= 0:
                        a = TCH * jj + los[jj]
                        nc.vector.tensor_mul(
                            ext[:, a : a + 128], ext[:, a : a + 128], mask_t[:]
                        )
                return ext, los

            def emit_pv(tch, jp, ext, los):
                jmax = 4 * tch + 3
                for jj in range(2):
                    j = jp + jj
                    lo = los[jj]
                    nc.tensor.matmul(
                        accs[tch][:, lo:TCH] if j > 0 else accs[tch][:, :],
                        v1[j][:],
                        ext[:, TCH * jj + lo : TCH * (jj + 1)],
                        start=(j == 0),
                        stop=(j == jmax),
                        skip_group_check=True,
                    )

            def emit_epilogue(tch):
                # ======== normalize + transpose + DMA out for chunk tch ====
                oT = epip.tile([65, TCH], f32, tag="oT", name=f"oT{tch}")
                nc.vector.tensor_copy(oT[:], accs[tch][:])
                ot = epip.tile([128, 4, H], f32, tag="ot", name=f"ot{tch}")
                for i in range(4):
                    Pe = miscp.tile([128, 65], f32, tag="misc", name=f"Pe{tch}_{i}")
                    nc.tensor.transpose(
                        Pe[:, :],
                        oT[:, 128 * i : 128 * (i + 1)],
                        idf_t[0:65, 0:65],
                    )
                    rec = epip.tile([128, 1], f32, tag="rec", name=f"rec{tch}_{i}")
                    nc.vector.reciprocal(rec[:], Pe[:, 64:65])
                    nc.vector.tensor_scalar_mul(ot[:, i, :], Pe[:, 0:64], rec[:])
                r0 = TCH * tch
                nc.gpsimd.dma_start(
                    out=out_d[r0 : r0 + TCH, :].rearrange("(i p) h -> p i h", i=4),
                    in_=ot[:],
                )

            # ---- the global pair stream ----
            slots = [
                (tch, jp) for tch in range(N_CHUNK) for jp in range(0, 4 * tch + 4, 2)
            ]
            slot_of = {p: k for k, p in enumerate(slots)}
            # paced projection queues with due slots
            queues = []  # (due_slot, items)
            qk0, v0 = proj_steps(0)
            for s in qk0:
                s()
            queues.append([slot_of[(0, 0)] + LAG, v0])
            for c in range(1, N_CHUNK):
                qkc, vc = proj_steps(c)
                queues.append([slot_of[(c, 0)], qkc])
                fd = (2 * c, 2 * c + 1)  # first diagonal pair index range
                queues.append([min(slot_of[(c, fd[0] * 2)] + LAG, len(slots)), vc])

            accs = {
                tch: accp.tile([65, TCH], f32, tag="acc", name=f"acc{tch}")
                for tch in range(N_CHUNK)
            }

            exts = {}
            done_chunk_pv = {}

            def drain_due(k):
                for q in queues:
                    due, items = q
                    if not items or due - k > 8:  # not urgent yet
                        continue
                    left = max(1, due - k)
                    n = -(-len(items) // left)
                    for _ in range(n):
                        if items:
                            items.pop(0)()

            def force_drain(idx):
                due, items = queues[idx]
                while items:
                    items.pop(0)()

            for k, (tch, jp) in enumerate(slots):
                # hard guard: chunk's qk projection before its first scores
                if jp == 0 and tch >= 1:
                    force_drain(2 * tch - 1)
                exts[k] = emit_scores_exp(tch, jp)
                if k == 0:
                    # keep the first two score pairs adjacent; no pops yet
                    continue
                kv = k - LAG
                if kv >= 0:
                    vt, vjp = slots[kv]
                    if vjp >= 4 * vt:  # diagonal pair: needs own chunk's v1
                        force_drain(2 * vt if vt >= 1 else 0)
                    emit_pv(vt, vjp, *exts.pop(kv))
                    if vjp == 4 * vt + 2:  # last pair of chunk vt
                        emit_epilogue(vt)
                drain_due(k)

            # trailing PVs + last epilogue
            for kv in range(len(slots) - LAG, len(slots)):
                vt, vjp = slots[kv]
                if vjp >= 4 * vt:
                    force_drain(2 * vt if vt >= 1 else 0)
                emit_pv(vt, vjp, *exts.pop(kv))
                if vjp == 4 * vt + 2:
                    emit_epilogue(vt)

    nc.compile()
    return nc


def _get_nc():
    if "nc" not in _CACHE:
        _CACHE["nc"] = _build()
    return _CACHE["nc"]


def _tile_w(w):
    """[C, F] -> [128, N_CT*F] with c-tile-major column blocks."""
    Cdim, F = w.shape
    return np.ascontiguousarray(
        w.reshape(Cdim // 128, 128, F).transpose(1, 0, 2).reshape(128, -1)
    )


def _host_inputs(x, w_q, w_k, w_v):
    bf = ml_dtypes.bfloat16
    x = np.asarray(x, dtype=np.float32)
    wqk = np.concatenate(
        [np.asarray(w_q, np.float32), np.asarray(w_k, np.float32)], 1
    )
    wv = np.asarray(w_v, np.float32)
    wqk_tiled = _tile_w(wqk).astype(bf)
    wv_tiled = _tile_w(wv).astype(bf)
    # multiplicative causal mask for transposed-score diag blocks: keep s <= t
    mask01 = np.triu(np.ones((128, 128), np.float32)).astype(bf)
    idf = np.eye(128, dtype=np.float32)
    idb = np.eye(128, dtype=np.float32).astype(bf)
    in_maps = []
    for i in range(N_CORES):
        # x^T pre-tiled: [128, chunk, c-tile, t] flattened per partition
        xT = np.ascontiguousarray(x[i].T).astype(bf)  # [C, T]
        xT4 = xT.reshape(N_CT, 128, N_CHUNK, TCH)     # [c, p, chunk, t]
        xTt = xT4.transpose(1, 2, 0, 3).reshape(128, N_CHUNK, -1)  # [p, chunk, c*t]
        blob0 = np.ascontiguousarray(
            np.concatenate(
                [wv_tiled, wqk_tiled, xTt[:, 0, :], xTt[:, 1, :]], axis=1
            )
        )
        xrest = np.ascontiguousarray(xTt[:, 2:, :].reshape(128, -1))
        in_maps.append(
            {
                "blob0": blob0,
                "xrest": xrest,
                "mask01": mask01,
                "idf": idf,
                "idb": idb,
            }
        )
    return in_maps


def run(x, w_q, w_k, w_v, trace=False, **trace_kwargs):
    from concourse.bass_utils import run_bass_kernel_spmd

    nc = _get_nc()
    in_maps = _host_inputs(x, w_q, w_k, w_v)
    res = run_bass_kernel_spmd(
        nc, in_maps, core_ids=list(range(N_CORES)), trace=trace, **trace_kwargs
    )
    out = np.stack([np.asarray(res.results[i]["out"]) for i in range(N_CORES)])
    return out.astype(np.float32), res


def kernel(x, w_q, w_k, w_v):
    out, _ = run(x, w_q, w_k, w_v, trace=False)
    return out


# revision 20
# speedup vs baseline: 1.1594x; 1.0106x over previous
"""Distributed Trainium2 kernel for a single attention head.

Problem: x:[8,2048,1024] f32, w_q/w_k/w_v:[1024,64] f32
  q,k,v = x@w ; scores = (q k^T)/sqrt(1024) causal-masked; out = softmax(scores)@v

Sharding: data-parallel over batch B=8 across the 8 NeuronCores (one batch
element per core, weights replicated, no collectives).

Per-core dataflow (T=2048, C=1024, H=64), built around keeping the ScalarE
exp stream (the serial resource: 20 x ~1.1us activations) gap-free:

  - host ships blob0 = [wv | wqk | x^T chunk 0] bf16 as ONE DMA at the head
    of the sync HWDGE queue, then x^T chunks 1-3 (one DMA each; HBM is
    shared with the sibling core, so the head transfer is kept minimal).
    q/k partition-duplication DMAs ride the gpsimd SWDGE queue; output DMAs
    ride sync behind the input stream.
  - 16 N=512 warm-up matmuls bridge the PE from the preamble to the blob0
    arrival so the HAM clock-gate is at 8/8 when real work starts.
  - projections with weights stationary (bf16): qkT [128,T] (q rows 0:64,
    k rows 64:128) and vT [64,T] per chunk.
  - scores computed TRANSPOSED per s-tile pair: S[s,t] = kT_slice.T @ qT
    (K=64), written to f32 PSUM tiles [128,1024] (two banks each, Sp
    bufs=2), so two score pairs can be banked ahead of the exp stream.
  - chunks 0-1 run scores unpacked on partition half 0 (no dup round-trip on
    the critical path); chunks 2-3 row-packed 2x in PE row-groups 0/1.
  - exp on ScalarE, scale=1/32 folded in (|scores|<~2, no max needed), one
    [128,1024] activation per pair -> bf16 SBUF; table set pre-loaded by a
    dummy exp during the DMA phase.
  - causal: diagonal 128x128 blocks multiplied by a 0/1 mask on VectorE
    after the exp (keeps the PE free of mask matmuls).
  - PV: out^T[h,t] accumulated per chunk with lhsT = [v | 1] so row 64 is
    the softmax denominator (fused row-sum). PV for pair k is emitted LAG=4
    slots behind its scores, so the in-order PE stream never blocks scores
    production on exp completion.
  - projection work for later chunks is paced between pair slots with
    explicit due-dates (qk of chunk c before its first scores; v of chunk c
    before its first diagonal PV).
  - epilogue: TensorE transpose back to [t,h], reciprocal-multiply on
    VectorE, one combined [512,64] DMA out per chunk.
"""

import os
import sys

import numpy as np

for p in ("/opt/trn_rl_repo",):
    if p not in sys.path and os.path.isdir(p):
        sys.path.insert(0, p)

import ml_dtypes  # noqa: E402

B, T, C, H = 8, 2048, 1024, 64
N_CORES = 8
TCH = 512                  # t-chunk
N_CHUNK = T // TCH         # 4
N_CT = C // 128            # 8 contraction tiles
SCALE = float(C) ** -0.5   # 1/32
N_WARM = 11                # PE warm-up matmuls (N=512 each, ~0.43us cold)
LAG = 4                    # PV trails scores by this many pair slots

_CACHE = {}


def _build():
    """Build + compile the SPMD Bass graph (same graph on all 8 cores)."""
    import concourse.bass as bass
    import concourse.mybir as mybir
    import concourse.tile as tile
    from concourse import bacc

    f32 = mybir.dt.float32
    bf16 = mybir.dt.bfloat16
    EXP = mybir.ActivationFunctionType.Exp

    nc = bacc.Bacc(
        "TRN2", target_bir_lowering=False, debug=False, num_devices=N_CORES
    )

    # blob0 = [wv | wqk | x^T chunk0 | x^T chunk1], x pre-tiled [c-tile][t]
    W_WV = N_CT * H
    W_WQK = N_CT * 128
    W_X = N_CT * TCH
    BLOB0_W = W_WV + W_WQK + W_X
    blob0_d = nc.dram_tensor("blob0", [128, BLOB0_W], bf16, kind="ExternalInput")
    xrest_d = nc.dram_tensor("xrest", [128, 3 * W_X], bf16, kind="ExternalInput")
    mask_d = nc.dram_tensor("mask01", [128, 128], bf16, kind="ExternalInput")
    idf_d = nc.dram_tensor("idf", [128, 128], f32, kind="ExternalInput")
    idb_d = nc.dram_tensor("idb", [128, 128], bf16, kind="ExternalInput")
    out_d = nc.dram_tensor("out", [T, H], f32, kind="ExternalOutput")

    with tile.TileContext(nc) as tc:
        with (
            tc.tile_pool(name="const", bufs=1) as constp,
            tc.tile_pool(name="xTp", bufs=1) as xTp,
            tc.tile_pool(name="qkp", bufs=1) as qkp,
            tc.tile_pool(name="q2p", bufs=2) as q2p,
            tc.tile_pool(name="vTp", bufs=2) as vTp,
            tc.tile_pool(name="v1p", bufs=1) as v1p,
            tc.tile_pool(name="exp", bufs=LAG + 2) as expp,
            tc.tile_pool(name="epi", bufs=2) as epip,
            tc.tile_pool(name="Sp", bufs=2, space="PSUM") as Sp,
            tc.tile_pool(name="accp", bufs=1, space="PSUM") as accp,
            tc.tile_pool(name="miscp", bufs=3, space="PSUM") as miscp,
        ):
            # ---- PE warm-up scratch ----
            warm_sb = constp.tile([128, TCH], bf16, tag="warm_sb", name="warm_sb")
            nc.vector.memset(warm_sb[:], 0.0)
            warm_act = constp.tile([128, 8], bf16, tag="warm_act", name="warm_act")
            warm_ps = miscp.tile([128, TCH], f32, tag="misc", name="warm_ps")
            for i in range(N_WARM):
                nc.tensor.matmul(
                    warm_ps[:, :],
                    warm_sb[:, 0:128],
                    warm_sb[:, :],
                    start=True,
                    stop=True,
                    skip_group_check=True,
                )

            # ---- input DMAs: all on the sync HWDGE queue, strict FIFO ----
            blob0_t = constp.tile([128, BLOB0_W], bf16, tag="blob0", name="blob0_t")
            nc.sync.dma_start(out=blob0_t[:], in_=blob0_d[:])
            wv_t = blob0_t[:, 0:W_WV].rearrange("p (n m) -> p n m", n=N_CT)
            wqk_t = blob0_t[:, W_WV : W_WV + W_WQK].rearrange(
                "p (n m) -> p n m", n=N_CT
            )
            a = W_WV + W_WQK
            xt = {0: blob0_t[:, a : a + W_X].rearrange("p (n m) -> p n m", n=N_CT)}
            xr_v = xrest_d[:].rearrange("p (t n m) -> p t n m", t=3, n=N_CT)
            for t in (1, 2, 3):
                xx = xTp.tile([128, N_CT, TCH], bf16, tag=f"x{t}", name=f"x{t}")
                nc.sync.dma_start(out=xx[:], in_=xr_v[:, t - 1, :, :])
                xt[t] = xx[:]

            # dummy exp: forces the ACT table-set load during the DMA phase
            nc.scalar.activation(warm_act[:], warm_sb[:, 0:8], EXP, scale=1.0)

            # ---- small constants on the gpsimd SWDGE queue ----
            mask_t = constp.tile([128, 128], bf16, tag="mask", name="mask_t")
            nc.gpsimd.dma_start(out=mask_t[:], in_=mask_d[:])
            idb_t = constp.tile([128, 128], bf16, tag="idb", name="idb_t")
            nc.gpsimd.dma_start(out=idb_t[:], in_=idb_d[:])
            idf_t = constp.tile([128, 128], f32, tag="idf", name="idf_t")
            nc.gpsimd.dma_start(out=idf_t[:], in_=idf_d[:])

            qk2 = {}   # [128, TCH] bf16 per chunk: qT (dup'd for chunks 2-3)
            kk2 = {}   # [128, TCH] bf16 per chunk: kT dup'd on both halves
            v1 = {}    # [128, 65] bf16 per s-tile: [v | 1]

            def proj_steps(tch):
                """Emission thunks for chunk `tch`: (qk steps, v steps)."""
                qk_steps = []
                v_steps = []
                state = {}

                def qk_mm(c):
                    def f():
                        if c == 0:
                            state["S"] = miscp.tile(
                                [128, TCH], f32, tag="misc", name=f"Sqk{tch}"
                            )
                        nc.tensor.matmul(
                            state["S"][:, :],
                            wqk_t[:, c, :],
                            xt[tch][:, c, :],
                            start=(c == 0),
                            stop=(c == N_CT - 1),
                            skip_group_check=True,
                        )
                    return f

                def qk_out():
                    S = state["S"]
                    q2 = q2p.tile([128, TCH], bf16, tag="q2", name=f"q2_{tch}")
                    k2 = qkp.tile([128, TCH], bf16, tag=f"k2_{tch}", name=f"k2_{tch}")
                    nc.vector.tensor_copy(k2[0:64, :], S[64:128, :])
                    nc.vector.tensor_copy(q2[0:64, :], S[0:64, :])
                    # chunks 0-1 score unpacked on half 0: q never needs the
                    # SWDGE dup round-trip; k still does (chunks 2-3 pack).
                    if tch > 1:
                        nc.gpsimd.dma_start(out=q2[64:128, :], in_=q2[0:64, :])
                    nc.gpsimd.dma_start(out=k2[64:128, :], in_=k2[0:64, :])
                    qk2[tch] = q2
                    kk2[tch] = k2

                def v_mm(c):
                    def f():
                        if c == 0:
                            state["Pv"] = miscp.tile(
                                [64, TCH], f32, tag="misc", name=f"Pv{tch}"
                            )
                        nc.tensor.matmul(
                            state["Pv"][:, :],
                            wv_t[:, c, :],
                            xt[tch][:, c, :],
                            start=(c == 0),
                            stop=(c == N_CT - 1),
                            skip_group_check=True,
                        )
                    return f

                def v_out():
                    vTt = vTp.tile([64, TCH], bf16, tag="vT", name=f"vT{tch}")
                    nc.vector.tensor_copy(vTt[:], state["Pv"][:, :])
                    state["vT"] = vTt

                def v1_build(i):
                    def f():
                        j = 4 * tch + i
                        Pt = miscp.tile([128, H], bf16, tag="misc", name=f"Pt{j}")
                        nc.tensor.transpose(
                            Pt[:, :],
                            state["vT"][:, 128 * i : 128 * (i + 1)],
                            idb_t[0:64, 0:64],
                        )
                        v1t = v1p.tile([128, 65], bf16, tag=f"v1_{j}", name=f"v1_{j}")
                        nc.vector.tensor_copy(v1t[:, 0:64], Pt[:, :])
                        nc.vector.memset(v1t[:, 64:65], 1.0)
                        v1[j] = v1t
                    return f

                for c in range(N_CT):
                    qk_steps.append(qk_mm(c))
                qk_steps.append(qk_out)
                for c in range(N_CT):
                    v_steps.append(v_mm(c))
                v_steps.append(v_out)
                for i in range(4):
                    v_steps.append(v1_build(i))
                return qk_steps, v_steps

            def emit_scores_exp(tch, jp):
                """Scores matmuls + exp + masks for pair (jp, jp+1)."""
                unpacked = tch <= 1 or (tch == 2 and jp <= 2)
                S2 = Sp.tile([128, 2 * TCH], f32, tag="S", name=f"S{tch}_{jp}")
                los = {}
                for jj in range(2):
                    j = jp + jj
                    rel = j - 4 * tch
                    lo = 128 * max(0, rel)
                    los[jj] = lo
                    half = slice(0, 64) if unpacked else slice(64 * jj, 64 * (jj + 1))
                    ksl = kk2[j // 4][half, 128 * (j % 4) : 128 * (j % 4 + 1)]
                    nc.tensor.matmul(
                        S2[:, TCH * jj + lo : TCH * (jj + 1)],
                        ksl,
                        qk2[tch][half, lo:TCH],
                        start=True,
                        stop=True,
                        skip_group_check=True,
                    )
                ext = expp.tile([128, 2 * TCH], bf16, tag="ex", name=f"ex{tch}_{jp}")
                nc.scalar.activation(ext[:], S2[:], EXP, scale=SCALE)
                # causal 0/1 mask on the diagonal 128x128 blocks (VectorE)
                for jj in range(2):
                    if jp + jj - 4 * tch >= 0:
                        a = TCH * jj + los[jj]
                        nc.vector.tensor_mul(
                            ext[:, a : a + 128], ext[:, a : a + 128], mask_t[:]
                        )
                return ext, los

            def emit_pv(tch, jp, ext, los):
                jmax = 4 * tch + 3
                for jj in range(2):
                    j = jp + jj
                    lo = los[jj]
                    nc.tensor.matmul(
                        accs[tch][:, lo:TCH] if j > 0 else accs[tch][:, :],
                        v1[j][:],
                        ext[:, TCH * jj + lo : TCH * (jj + 1)],
                        start=(j == 0),
                        stop=(j == jmax),
                        skip_group_check=True,
                    )

            def emit_epilogue(tch):
                # ======== normalize + transpose + DMA out for chunk tch ====
                oT = epip.tile([65, TCH], f32, tag="oT", name=f"oT{tch}")
                nc.vector.tensor_copy(oT[:], accs[tch][:])
                ot = epip.tile([128, 4, H], f32, tag="ot", name=f"ot{tch}")
                for i in range(4):
                    Pe = miscp.tile([128, 65], f32, tag="misc", name=f"Pe{tch}_{i}")
                    nc.tensor.transpose(
                        Pe[:, :],
                        oT[:, 128 * i : 128 * (i + 1)],
                        idf_t[0:65, 0:65],
                    )
                    rec = epip.tile([128, 1], f32, tag="rec", name=f"rec{tch}_{i}")
                    nc.vector.reciprocal(rec[:], Pe[:, 64:65])
                    nc.vector.tensor_scalar_mul(ot[:, i, :], Pe[:, 0:64], rec[:])
                r0 = TCH * tch
                nc.sync.dma_start(
                    out=out_d[r0 : r0 + TCH, :].rearrange("(i p) h -> p i h", i=4),
                    in_=ot[:],
                )

            # ---- the global pair stream ----
            slots = [
                (tch, jp) for tch in range(N_CHUNK) for jp in range(0, 4 * tch + 4, 2)
            ]
            slot_of = {p: k for k, p in enumerate(slots)}
            # paced projection queues with due slots
            queues = []  # (due_slot, items)
            qk0, v0 = proj_steps(0)
            for s in qk0:
                s()
            queues.append([slot_of[(0, 0)] + LAG, v0])
            for c in range(1, N_CHUNK):
                qkc, vc = proj_steps(c)
                queues.append([slot_of[(c, 0)], qkc])
                fd = (2 * c, 2 * c + 1)  # first diagonal pair index range
                queues.append([min(slot_of[(c, fd[0] * 2)] + LAG, len(slots)), vc])

            accs = {
                tch: accp.tile([65, TCH], f32, tag="acc", name=f"acc{tch}")
                for tch in range(N_CHUNK)
            }

            exts = {}
            done_chunk_pv = {}

            def drain_due(k):
                for q in queues:
                    due, items = q
                    if not items or due - k > 8:  # not urgent yet
                        continue
                    left = max(1, due - k)
                    n = -(-len(items) // left)
                    for _ in range(n):
                        if items:
                            items.pop(0)()

            def force_drain(idx):
                due, items = queues[idx]
                while items:
                    items.pop(0)()

            for k, (tch, jp) in enumerate(slots):
                # hard guard: chunk's qk projection before its first scores
                if jp == 0 and tch >= 1:
                    force_drain(2 * tch - 1)
                exts[k] = emit_scores_exp(tch, jp)
                if k == 0:
                    # keep the first two score pairs adjacent; no pops yet
                    continue
                kv = k - LAG
                if kv >= 0:
                    vt, vjp = slots[kv]
                    if vjp >= 4 * vt:  # diagonal pair: needs own chunk's v1
                        force_drain(2 * vt if vt >= 1 else 0)
                    emit_pv(vt, vjp, *exts.pop(kv))
                    if vjp == 4 * vt + 2:  # last pair of chunk vt
                        emit_epilogue(vt)
                drain_due(k)

            # trailing PVs + last epilogue
            for kv in range(len(slots) - LAG, len(slots)):
                vt, vjp = slots[kv]
                if vjp >= 4 * vt:
                    force_drain(2 * vt if vt >= 1 else 0)
                emit_pv(vt, vjp, *exts.pop(kv))
                if vjp == 4 * vt + 2:
                    emit_epilogue(vt)

    nc.compile()
    return nc


def _get_nc():
    if "nc" not in _CACHE:
        _CACHE["nc"] = _build()
    return _CACHE["nc"]


def _tile_w(w):
    """[C, F] -> [128, N_CT*F] with c-tile-major column blocks."""
    Cdim, F = w.shape
    return np.ascontiguousarray(
        w.reshape(Cdim // 128, 128, F).transpose(1, 0, 2).reshape(128, -1)
    )


def _host_inputs(x, w_q, w_k, w_v):
    bf = ml_dtypes.bfloat16
    x = np.asarray(x, dtype=np.float32)
    wqk = np.concatenate(
        [np.asarray(w_q, np.float32), np.asarray(w_k, np.float32)], 1
    )
    wv = np.asarray(w_v, np.float32)
    wqk_tiled = _tile_w(wqk).astype(bf)
    wv_tiled = _tile_w(wv).astype(bf)
    # multiplicative causal mask for transposed-score diag blocks: keep s <= t
    mask01 = np.triu(np.ones((128, 128), np.float32)).astype(bf)
    idf = np.eye(128, dtype=np.float32)
    idb = np.eye(128, dtype=np.float32).astype(bf)
    in_maps = []
    for i in range(N_CORES):
        # x^T pre-tiled: [128, chunk, c-tile, t] flattened per partition
        xT = np.ascontiguousarray(x[i].T).astype(bf)  # [C, T]
        xT4 = xT.reshape(N_CT, 128, N_CHUNK, TCH)     # [c, p, chunk, t]
        xTt = xT4.transpose(1, 2, 0, 3).reshape(128, N_CHUNK, -1)  # [p, chunk, c*t]
        blob0 = np.ascontiguousarray(
            np.concatenate([wv_tiled, wqk_tiled, xTt[:, 0, :]], axis=1)
        )
        xrest = np.ascontiguousarray(xTt[:, 1:, :].reshape(128, -1))
        in_maps.append(
            {
                "blob0": blob0,
                "xrest": xrest,
                "mask01": mask01,
                "idf": idf,
                "idb": idb,
            }
        )
    return in_maps


def run(x, w_q, w_k, w_v, trace=False, **trace_kwargs):
    from concourse.bass_utils import run_bass_kernel_spmd

    nc = _get_nc()
    in_maps = _host_inputs(x, w_q, w_k, w_v)
    res = run_bass_kernel_spmd(
        nc, in_maps, core_ids=list(range(N_CORES)), trace=trace, **trace_kwargs
    )
    out = np.stack([np.asarray(res.results[i]["out"]) for i in range(N_CORES)])
    return out.astype(np.float32), res


def kernel(x, w_q, w_k, w_v):
    out, _ = run(x, w_q, w_k, w_v, trace=False)
    return out


# revision 24
# speedup vs baseline: 1.3176x; 1.1365x over previous
"""Distributed Trainium2 kernel for a single attention head.

Problem: x:[8,2048,1024] f32, w_q/w_k/w_v:[1024,64] f32
  q,k,v = x@w ; scores = (q k^T)/sqrt(1024) causal-masked; out = softmax(scores)@v

Sharding: data-parallel over batch B=8 across the 8 NeuronCores (one batch
element per core, weights replicated, no collectives).

Per-core dataflow (T=2048, C=1024, H=64), built around keeping the ScalarE
exp stream (the serial resource: 20 x ~1.1us activations) gap-free:

  - host ships blob0 = [wv | wqk | x^T chunk 0] bf16 as ONE DMA at the head
    of the sync HWDGE queue, then x^T chunks 1-3 (one DMA each; HBM is
    shared with the sibling core, so the head transfer is kept minimal).
    q/k partition-duplication DMAs ride the gpsimd SWDGE queue; output DMAs
    ride sync behind the input stream.
  - 16 N=512 warm-up matmuls bridge the PE from the preamble to the blob0
    arrival so the HAM clock-gate is at 8/8 when real work starts.
  - projections with weights stationary (bf16): qkT [128,T] (q rows 0:64,
    k rows 64:128) and vT [64,T] per chunk.
  - scores computed TRANSPOSED per s-tile pair: S[s,t] = kT_slice.T @ qT
    (K=64), written to f32 PSUM tiles [128,1024] (two banks each, Sp
    bufs=2), so two score pairs can be banked ahead of the exp stream.
  - chunks 0-1 run scores unpacked on partition half 0 (no dup round-trip on
    the critical path); chunks 2-3 row-packed 2x in PE row-groups 0/1.
  - exp on ScalarE, scale=1/32 folded in (|scores|<~2, no max needed), one
    [128,1024] activation per pair -> bf16 SBUF; table set pre-loaded by a
    dummy exp during the DMA phase.
  - causal: diagonal 128x128 blocks multiplied by a 0/1 mask on VectorE
    after the exp (keeps the PE free of mask matmuls).
  - PV: out^T[h,t] accumulated per chunk with lhsT = [v | 1] so row 64 is
    the softmax denominator (fused row-sum). PV for pair k is emitted LAG=4
    slots behind its scores, so the in-order PE stream never blocks scores
    production on exp completion.
  - projection work for later chunks is paced between pair slots with
    explicit due-dates (qk of chunk c before its first scores; v of chunk c
    before its first diagonal PV).
  - epilogue: TensorE transpose back to [t,h], reciprocal-multiply on
    VectorE, one combined [512,64] DMA out per chunk.
"""

import os
import sys

import numpy as np

for p in ("/opt/trn_rl_repo",):
    if p not in sys.path and os.path.isdir(p):
        sys.path.insert(0, p)

import ml_dtypes  # noqa: E402

B, T, C, H = 8, 2048, 1024, 64
N_CORES = 8
TCH = 512                  # t-chunk
N_CHUNK = T // TCH         # 4
N_CT = C // 128            # 8 contraction tiles
SCALE = float(C) ** -0.5   # 1/32
N_WARM = 11                # PE warm-up matmuls (N=512 each, ~0.43us cold)
LAG = 4                    # PV trails scores by this many pair slots

_CACHE = {}


def _build():
    """Build + compile the SPMD Bass graph (same graph on all 8 cores)."""
    import concourse.bass as bass
    import concourse.mybir as mybir
    import concourse.tile as tile
    from concourse import bacc

    f32 = mybir.dt.float32
    bf16 = mybir.dt.bfloat16
    EXP = mybir.ActivationFunctionType.Exp

    nc = bacc.Bacc(
        "TRN2", target_bir_lowering=False, debug=False, num_devices=N_CORES
    )

    # blob0 = [wv | wqk | x^T chunk0 | x^T chunk1], x pre-tiled [c-tile][t]
    W_WV = N_CT * H
    W_WQK = N_CT * 128
    W_X = N_CT * TCH
    BLOB0_W = W_WV + W_WQK + W_X
    blob0_d = nc.dram_tensor("blob0", [128, BLOB0_W], bf16, kind="ExternalInput")
    xrest_d = nc.dram_tensor("xrest", [128, 3 * W_X], bf16, kind="ExternalInput")
    mask_d = nc.dram_tensor("mask01", [128, 128], bf16, kind="ExternalInput")
    idf_d = nc.dram_tensor("idf", [128, 128], f32, kind="ExternalInput")
    idb_d = nc.dram_tensor("idb", [128, 128], bf16, kind="ExternalInput")
    out_d = nc.dram_tensor("out", [T, H], f32, kind="ExternalOutput")

    with tile.TileContext(nc) as tc:
        with (
            tc.tile_pool(name="const", bufs=1) as constp,
            tc.tile_pool(name="xTp", bufs=1) as xTp,
            tc.tile_pool(name="qkp", bufs=1) as qkp,
            tc.tile_pool(name="q2p", bufs=2) as q2p,
            tc.tile_pool(name="vTp", bufs=2) as vTp,
            tc.tile_pool(name="v1p", bufs=1) as v1p,
            tc.tile_pool(name="exp", bufs=LAG + 2) as expp,
            tc.tile_pool(name="epi", bufs=2) as epip,
            tc.tile_pool(name="Sp", bufs=2, space="PSUM") as Sp,
            tc.tile_pool(name="accp", bufs=1, space="PSUM") as accp,
            tc.tile_pool(name="miscp", bufs=3, space="PSUM") as miscp,
        ):
            # ---- PE warm-up scratch ----
            warm_sb = constp.tile([128, TCH], bf16, tag="warm_sb", name="warm_sb")
            nc.vector.memset(warm_sb[:], 0.0)
            warm_act = constp.tile([128, 8], bf16, tag="warm_act", name="warm_act")
            warm_ps = miscp.tile([128, TCH], f32, tag="misc", name="warm_ps")
            for i in range(N_WARM):
                nc.tensor.matmul(
                    warm_ps[:, :],
                    warm_sb[:, 0:128],
                    warm_sb[:, :],
                    start=True,
                    stop=True,
                    skip_group_check=True,
                )

            # ---- input DMAs: all on the sync HWDGE queue, strict FIFO ----
            blob0_t = constp.tile([128, BLOB0_W], bf16, tag="blob0", name="blob0_t")
            nc.sync.dma_start(out=blob0_t[:], in_=blob0_d[:])
            wv_t = blob0_t[:, 0:W_WV].rearrange("p (n m) -> p n m", n=N_CT)
            wqk_t = blob0_t[:, W_WV : W_WV + W_WQK].rearrange(
                "p (n m) -> p n m", n=N_CT
            )
            a = W_WV + W_WQK
            xt = {0: blob0_t[:, a : a + W_X].rearrange("p (n m) -> p n m", n=N_CT)}
            xr_v = xrest_d[:].rearrange("p (t n m) -> p t n m", t=3, n=N_CT)
            for t in (1, 2, 3):
                xx = xTp.tile([128, N_CT, TCH], bf16, tag=f"x{t}", name=f"x{t}")
                nc.sync.dma_start(out=xx[:], in_=xr_v[:, t - 1, :, :])
                xt[t] = xx[:]

            # dummy exp: forces the ACT table-set load during the DMA phase
            nc.scalar.activation(warm_act[:], warm_sb[:, 0:8], EXP, scale=1.0)

            # ---- small constants on the gpsimd SWDGE queue ----
            mask_t = constp.tile([128, 128], bf16, tag="mask", name="mask_t")
            nc.gpsimd.dma_start(out=mask_t[:], in_=mask_d[:])
            idb_t = constp.tile([128, 128], bf16, tag="idb", name="idb_t")
            nc.gpsimd.dma_start(out=idb_t[:], in_=idb_d[:])
            idf_t = constp.tile([128, 128], f32, tag="idf", name="idf_t")
            nc.gpsimd.dma_start(out=idf_t[:], in_=idf_d[:])

            qk2 = {}   # [128, TCH] bf16 per chunk: qT (dup'd for chunks 2-3)
            kk2 = {}   # [128, TCH] bf16 per chunk: kT dup'd on both halves
            v1 = {}    # [128, 65] bf16 per s-tile: [v | 1]

            def qk_steps(tch):
                """Emission thunks for chunk `tch`'s q/k projection."""
                steps = []
                state = {}

                def qk_mm(c):
                    def f():
                        if c == 0:
                            state["S"] = miscp.tile(
                                [128, TCH], f32, tag="misc", name=f"Sqk{tch}"
                            )
                        nc.tensor.matmul(
                            state["S"][:, :],
                            wqk_t[:, c, :],
                            xt[tch][:, c, :],
                            start=(c == 0),
                            stop=(c == N_CT - 1),
                            skip_group_check=True,
                        )
                    return f

                def qk_out():
                    S = state["S"]
                    q2 = q2p.tile([128, TCH], bf16, tag="q2", name=f"q2_{tch}")
                    k2 = qkp.tile([128, TCH], bf16, tag=f"k2_{tch}", name=f"k2_{tch}")
                    # q first: the chunk's own scores need it immediately
                    nc.vector.tensor_copy(q2[0:64, :], S[0:64, :])
                    nc.vector.tensor_copy(k2[0:64, :], S[64:128, :])
                    # chunks 0-1 (and chunk 2's first pairs) score unpacked on
                    # half 0: q needs no SWDGE dup round-trip on the critical
                    # path; k still does (used packed by chunks 2-3).
                    if tch > 1:
                        nc.gpsimd.dma_start(out=q2[64:128, :], in_=q2[0:64, :])
                    nc.gpsimd.dma_start(out=k2[64:128, :], in_=k2[0:64, :])
                    qk2[tch] = q2
                    kk2[tch] = k2

                for c in range(N_CT):
                    steps.append(qk_mm(c))
                steps.append(qk_out)
                return steps

            def v1_build(state, which, tch, i):
                def f():
                    j = 4 * tch + i
                    Pt = miscp.tile([128, H], bf16, tag="misc", name=f"Pt{j}")
                    if which == 0:
                        nc.tensor.transpose(
                            Pt[:, :],
                            state["vTa"][:, 128 * i : 128 * (i + 1)],
                            idb_t[0:64, 0:64],
                        )
                    else:
                        nc.tensor.transpose(
                            Pt[:, :],
                            state["vTb"][64:128, 128 * i : 128 * (i + 1)],
                            idb_t[64:128, 64:128],
                        )
                    v1t = v1p.tile([128, 65], bf16, tag=f"v1_{j}", name=f"v1_{j}")
                    nc.vector.tensor_copy(v1t[:, 0:64], Pt[:, :])
                    nc.vector.memset(v1t[:, 64:65], 1.0)
                    v1[j] = v1t
                return f

            def vpair_steps(ca, cb):
                """v projections for chunks ca/cb column-packed 2x on the PE
                (ca in PE column group 0 -> PSUM partitions 0:64, cb in
                column group 1 -> partitions 64:128), plus the [v|1] builds.
                Returns (head_steps, buildB_steps): buildB (chunk cb's v1)
                has a later deadline."""
                head = []
                state = {}

                def v_mm(c):
                    def f():
                        if c == 0:
                            state["PvA"] = miscp.tile(
                                [64, TCH], f32, tag="misc", name=f"PvA{ca}"
                            )
                            state["PvB"] = miscp.tile(
                                [128, TCH], f32, tag="misc", name=f"PvB{cb}"
                            )
                        nc.tensor.matmul(
                            state["PvA"][:, :],
                            wv_t[:, c, :],
                            xt[ca][:, c, :],
                            start=(c == 0),
                            stop=(c == N_CT - 1),
                            skip_group_check=True,
                        )
                        nc.tensor.matmul(
                            state["PvB"][64:128, :],
                            wv_t[:, c, :],
                            xt[cb][:, c, :],
                            start=(c == 0),
                            stop=(c == N_CT - 1),
                            skip_group_check=True,
                        )
                    return f

                def v_out():
                    vTa = vTp.tile([64, TCH], bf16, tag="vTa", name=f"vT{ca}")
                    nc.vector.tensor_copy(vTa[:], state["PvA"][:, :])
                    vTb = vTp.tile([128, TCH], bf16, tag="vTb", name=f"vT{cb}")
                    nc.vector.tensor_copy(vTb[64:128, :], state["PvB"][64:128, :])
                    state["vTa"] = vTa
                    state["vTb"] = vTb

                for c in range(N_CT):
                    head.append(v_mm(c))
                head.append(v_out)
                for i in range(4):
                    head.append(v1_build(state, 0, ca, i))
                buildB = [v1_build(state, 1, cb, i) for i in range(4)]
                return head, buildB

            def emit_scores_exp(tch, jp):
                """Scores matmuls + exp + masks for pair (jp, jp+1)."""
                unpacked = tch <= 1 or (tch == 2 and jp <= 2)
                S2 = Sp.tile([128, 2 * TCH], f32, tag="S", name=f"S{tch}_{jp}")
                los = {}
                for jj in range(2):
                    j = jp + jj
                    rel = j - 4 * tch
                    lo = 128 * max(0, rel)
                    los[jj] = lo
                    half = slice(0, 64) if unpacked else slice(64 * jj, 64 * (jj + 1))
                    ksl = kk2[j // 4][half, 128 * (j % 4) : 128 * (j % 4 + 1)]
                    nc.tensor.matmul(
                        S2[:, TCH * jj + lo : TCH * (jj + 1)],
                        ksl,
                        qk2[tch][half, lo:TCH],
                        start=True,
                        stop=True,
                        skip_group_check=True,
                    )
                ext = expp.tile([128, 2 * TCH], bf16, tag="ex", name=f"ex{tch}_{jp}")
                nc.scalar.activation(ext[:], S2[:], EXP, scale=SCALE)
                return ext, los

            def emit_pv(tch, jp, ext, los):
                # causal 0/1 mask on the diagonal 128x128 blocks (VectorE),
                # deferred to PV-emission time so it never delays the q/k
                # copies in the DVE stream
                for jj in range(2):
                    if jp + jj - 4 * tch >= 0:
                        a = TCH * jj + los[jj]
                        nc.vector.tensor_mul(
                            ext[:, a : a + 128], ext[:, a : a + 128], mask_t[:]
                        )
                jmax = 4 * tch + 3
                for jj in range(2):
                    j = jp + jj
                    lo = los[jj]
                    nc.tensor.matmul(
                        accs[tch][:, lo:TCH] if j > 0 else accs[tch][:, :],
                        v1[j][:],
                        ext[:, TCH * jj + lo : TCH * (jj + 1)],
                        start=(j == 0),
                        stop=(j == jmax),
                        skip_group_check=True,
                    )

            def emit_epilogue(tch):
                # ======== normalize + transpose + DMA out for chunk tch ====
                oT = epip.tile([65, TCH], f32, tag="oT", name=f"oT{tch}")
                nc.vector.tensor_copy(oT[:], accs[tch][:])
                ot = epip.tile([128, 4, H], f32, tag="ot", name=f"ot{tch}")
                for i in range(4):
                    Pe = miscp.tile([128, 65], f32, tag="misc", name=f"Pe{tch}_{i}")
                    nc.tensor.transpose(
                        Pe[:, :],
                        oT[:, 128 * i : 128 * (i + 1)],
                        idf_t[0:65, 0:65],
                    )
                    rec = epip.tile([128, 1], f32, tag="rec", name=f"rec{tch}_{i}")
                    nc.vector.reciprocal(rec[:], Pe[:, 64:65])
                    nc.vector.tensor_scalar_mul(ot[:, i, :], Pe[:, 0:64], rec[:])
                r0 = TCH * tch
                nc.sync.dma_start(
                    out=out_d[r0 : r0 + TCH, :].rearrange("(i p) h -> p i h", i=4),
                    in_=ot[:],
                )

            # ---- the global pair stream ----
            slots = [
                (tch, jp) for tch in range(N_CHUNK) for jp in range(0, 4 * tch + 4, 2)
            ]
            slot_of = {p: k for k, p in enumerate(slots)}
            n_slots = len(slots)
            # paced projection queues with due slots (emitted between pair
            # slots; ordered dict so earlier-due queues pop first)
            for s in qk_steps(0):
                s()
            v01_head, v01_bB = vpair_steps(0, 1)
            v23_head, v23_bB = vpair_steps(2, 3)
            queues = {
                "qk1": [slot_of[(1, 0)], qk_steps(1)],
                "v01h": [slot_of[(0, 0)] + LAG, v01_head],
                "qk2": [slot_of[(2, 0)], qk_steps(2)],
                "v01b": [min(slot_of[(1, 4)] + LAG, n_slots), v01_bB],
                "qk3": [slot_of[(3, 0)], qk_steps(3)],
                "v23h": [min(slot_of[(2, 8)] + LAG, n_slots), v23_head],
                "v23b": [min(slot_of[(3, 12)] + LAG, n_slots), v23_bB],
            }

            accs = {
                tch: accp.tile([65, TCH], f32, tag="acc", name=f"acc{tch}")
                for tch in range(N_CHUNK)
            }

            exts = {}

            blocked_by = {"v01b": "v01h", "v23b": "v23h"}

            def drain_due(k):
                for name, q in queues.items():
                    due, items = q
                    if not items or due - k > 8:  # not urgent yet
                        continue
                    dep = blocked_by.get(name)
                    if dep and queues[dep][1]:  # build needs its head first
                        continue
                    left = max(1, due - k)
                    n = -(-len(items) // left)
                    for _ in range(n):
                        if items:
                            items.pop(0)()

            def force_drain(*names):
                for name in names:
                    items = queues[name][1]
                    while items:
                        items.pop(0)()

            def emit_pv_slot(kv):
                vt, vjp = slots[kv]
                if vjp >= 4 * vt:  # diagonal pair: needs own chunk's v1
                    if vt == 0:
                        force_drain("v01h")
                    elif vt == 1:
                        force_drain("v01h", "v01b")
                    elif vt == 2:
                        force_drain("v23h")
                    else:
                        force_drain("v23h", "v23b")
                emit_pv(vt, vjp, *exts.pop(kv))
                if vjp == 4 * vt + 2:  # last pair of chunk vt
                    emit_epilogue(vt)

            for k, (tch, jp) in enumerate(slots):
                # hard guard: chunk's qk projection before its first scores
                if jp == 0 and tch >= 1:
                    force_drain(f"qk{tch}")
                exts[k] = emit_scores_exp(tch, jp)
                if k == 0:
                    # keep the first two score pairs adjacent; no pops yet
                    continue
                if k - LAG >= 0:
                    emit_pv_slot(k - LAG)
                drain_due(k)

            # trailing PVs + last epilogue
            for kv in range(n_slots - LAG, n_slots):
                emit_pv_slot(kv)

    nc.compile()
    return nc


def _get_nc():
    if "nc" not in _CACHE:
        _CACHE["nc"] = _build()
    return _CACHE["nc"]


def _tile_w(w):
    """[C, F] -> [128, N_CT*F] with c-tile-major column blocks."""
    Cdim, F = w.shape
    return np.ascontiguousarray(
        w.reshape(Cdim // 128, 128, F).transpose(1, 0, 2).reshape(128, -1)
    )


def _host_inputs(x, w_q, w_k, w_v):
    bf = ml_dtypes.bfloat16
    x = np.asarray(x, dtype=np.float32)
    wqk = np.concatenate(
        [np.asarray(w_q, np.float32), np.asarray(w_k, np.float32)], 1
    )
    wv = np.asarray(w_v, np.float32)
    wqk_tiled = _tile_w(wqk).astype(bf)
    wv_tiled = _tile_w(wv).astype(bf)
    # multiplicative causal mask for transposed-score diag blocks: keep s <= t
    mask01 = np.triu(np.ones((128, 128), np.float32)).astype(bf)
    idf = np.eye(128, dtype=np.float32)
    idb = np.eye(128, dtype=np.float32).astype(bf)
    in_maps = []
    for i in range(N_CORES):
        # x^T pre-tiled: [128, chunk, c-tile, t] flattened per partition
        xT = np.ascontiguousarray(x[i].T).astype(bf)  # [C, T]
        xT4 = xT.reshape(N_CT, 128, N_CHUNK, TCH)     # [c, p, chunk, t]
        xTt = xT4.transpose(1, 2, 0, 3).reshape(128, N_CHUNK, -1)  # [p, chunk, c*t]
        blob0 = np.ascontiguousarray(
            np.concatenate([wv_tiled, wqk_tiled, xTt[:, 0, :]], axis=1)
        )
        xrest = np.ascontiguousarray(xTt[:, 1:, :].reshape(128, -1))
        in_maps.append(
            {
                "blob0": blob0,
                "xrest": xrest,
                "mask01": mask01,
                "idf": idf,
                "idb": idb,
            }
        )
    return in_maps


def run(x, w_q, w_k, w_v, trace=False, **trace_kwargs):
    from concourse.bass_utils import run_bass_kernel_spmd

    nc = _get_nc()
    in_maps = _host_inputs(x, w_q, w_k, w_v)
    res = run_bass_kernel_spmd(
        nc, in_maps, core_ids=list(range(N_CORES)), trace=trace, **trace_kwargs
    )
    out = np.stack([np.asarray(res.results[i]["out"]) for i in range(N_CORES)])
    return out.astype(np.float32), res


def kernel(x, w_q, w_k, w_v):
    out, _ = run(x, w_q, w_k, w_v, trace=False)
    return out


# revision 25
# speedup vs baseline: 1.3585x; 1.0311x over previous
"""Distributed Trainium2 kernel for a single attention head.

Problem: x:[8,2048,1024] f32, w_q/w_k/w_v:[1024,64] f32
  q,k,v = x@w ; scores = (q k^T)/sqrt(1024) causal-masked; out = softmax(scores)@v

Sharding: data-parallel over batch B=8 across the 8 NeuronCores (one batch
element per core, weights replicated, no collectives).

Per-core dataflow (T=2048, C=1024, H=64), built around keeping the ScalarE
exp stream (the serial resource: 20 x ~1.1us activations) gap-free:

  - host ships blob0 = [wv | wqk | x^T chunk 0] bf16 as ONE DMA at the head
    of the sync HWDGE queue, then x^T chunks 1-3 (one DMA each; HBM is
    shared with the sibling core, so the head transfer is kept minimal).
    q/k partition-duplication DMAs ride the gpsimd SWDGE queue; output DMAs
    ride sync behind the input stream.
  - 16 N=512 warm-up matmuls bridge the PE from the preamble to the blob0
    arrival so the HAM clock-gate is at 8/8 when real work starts.
  - projections with weights stationary (bf16): qkT [128,T] (q rows 0:64,
    k rows 64:128) and vT [64,T] per chunk.
  - scores computed TRANSPOSED per s-tile pair: S[s,t] = kT_slice.T @ qT
    (K=64), written to f32 PSUM tiles [128,1024] (two banks each, Sp
    bufs=2), so two score pairs can be banked ahead of the exp stream.
  - chunks 0-1 run scores unpacked on partition half 0 (no dup round-trip on
    the critical path); chunks 2-3 row-packed 2x in PE row-groups 0/1.
  - exp on ScalarE, scale=1/32 folded in (|scores|<~2, no max needed), one
    [128,1024] activation per pair -> bf16 SBUF; table set pre-loaded by a
    dummy exp during the DMA phase.
  - causal: diagonal 128x128 blocks multiplied by a 0/1 mask on VectorE
    after the exp (keeps the PE free of mask matmuls).
  - PV: out^T[h,t] accumulated per chunk with lhsT = [v | 1] so row 64 is
    the softmax denominator (fused row-sum). PV for pair k is emitted LAG=4
    slots behind its scores, so the in-order PE stream never blocks scores
    production on exp completion.
  - projection work for later chunks is paced between pair slots with
    explicit due-dates (qk of chunk c before its first scores; v of chunk c
    before its first diagonal PV).
  - epilogue: TensorE transpose back to [t,h], reciprocal-multiply on
    VectorE, one combined [512,64] DMA out per chunk.
"""

import os
import sys

import numpy as np

for p in ("/opt/trn_rl_repo",):
    if p not in sys.path and os.path.isdir(p):
        sys.path.insert(0, p)

import ml_dtypes  # noqa: E402

B, T, C, H = 8, 2048, 1024, 64
N_CORES = 8
TCH = 512                  # t-chunk
N_CHUNK = T // TCH         # 4
N_CT = C // 128            # 8 contraction tiles
SCALE = float(C) ** -0.5   # 1/32
N_WARM = 11                # PE warm-up matmuls (N=512 each, ~0.43us cold)
LAG = 4                    # PV trails scores by this many pair slots

_CACHE = {}


def _build():
    """Build + compile the SPMD Bass graph (same graph on all 8 cores)."""
    import concourse.bass as bass
    import concourse.mybir as mybir
    import concourse.tile as tile
    from concourse import bacc

    f32 = mybir.dt.float32
    bf16 = mybir.dt.bfloat16
    EXP = mybir.ActivationFunctionType.Exp

    nc = bacc.Bacc(
        "TRN2", target_bir_lowering=False, debug=False, num_devices=N_CORES
    )

    # blob0 = [wv | wqk | x^T chunk0 | x^T chunk1], x pre-tiled [c-tile][t]
    W_WV = N_CT * H
    W_WQK = N_CT * 128
    W_X = N_CT * TCH
    BLOB0_W = W_WV + W_WQK + W_X
    blob0_d = nc.dram_tensor("blob0", [128, BLOB0_W], bf16, kind="ExternalInput")
    xrest_d = nc.dram_tensor("xrest", [128, 3 * W_X], bf16, kind="ExternalInput")
    mask_d = nc.dram_tensor("mask01", [128, 128], bf16, kind="ExternalInput")
    idf_d = nc.dram_tensor("idf", [128, 128], f32, kind="ExternalInput")
    idb_d = nc.dram_tensor("idb", [128, 128], bf16, kind="ExternalInput")
    out_d = nc.dram_tensor("out", [T, H], f32, kind="ExternalOutput")

    with tile.TileContext(nc) as tc:
        with (
            tc.tile_pool(name="const", bufs=1) as constp,
            tc.tile_pool(name="xTp", bufs=1) as xTp,
            tc.tile_pool(name="qkp", bufs=1) as qkp,
            tc.tile_pool(name="q2p", bufs=2) as q2p,
            tc.tile_pool(name="vTp", bufs=2) as vTp,
            tc.tile_pool(name="v1p", bufs=1) as v1p,
            tc.tile_pool(name="exp", bufs=LAG + 2) as expp,
            tc.tile_pool(name="epi", bufs=2) as epip,
            tc.tile_pool(name="Sp", bufs=2, space="PSUM") as Sp,
            tc.tile_pool(name="accp", bufs=1, space="PSUM") as accp,
            tc.tile_pool(name="miscp", bufs=3, space="PSUM") as miscp,
        ):
            # ---- PE warm-up scratch ----
            warm_sb = constp.tile([128, TCH], bf16, tag="warm_sb", name="warm_sb")
            nc.vector.memset(warm_sb[:], 0.0)
            warm_act = constp.tile([128, 8], bf16, tag="warm_act", name="warm_act")
            warm_ps = miscp.tile([128, TCH], f32, tag="misc", name="warm_ps")
            for i in range(N_WARM):
                nc.tensor.matmul(
                    warm_ps[:, :],
                    warm_sb[:, 0:128],
                    warm_sb[:, :],
                    start=True,
                    stop=True,
                    skip_group_check=True,
                )

            # ---- input DMAs: all on the sync HWDGE queue, strict FIFO ----
            blob0_t = constp.tile([128, BLOB0_W], bf16, tag="blob0", name="blob0_t")
            HALF0 = W_WV + W_WQK + W_X // 2
            nc.sync.dma_start(out=blob0_t[:, 0:HALF0], in_=blob0_d[:, 0:HALF0])
            nc.sync.dma_start(out=blob0_t[:, HALF0:], in_=blob0_d[:, HALF0:])
            wv_t = blob0_t[:, 0:W_WV].rearrange("p (n m) -> p n m", n=N_CT)
            wqk_t = blob0_t[:, W_WV : W_WV + W_WQK].rearrange(
                "p (n m) -> p n m", n=N_CT
            )
            a = W_WV + W_WQK
            xt = {0: blob0_t[:, a : a + W_X].rearrange("p (n m) -> p n m", n=N_CT)}
            xr_v = xrest_d[:].rearrange("p (t n m) -> p t n m", t=3, n=N_CT)
            for t in (1, 2, 3):
                xx = xTp.tile([128, N_CT, TCH], bf16, tag=f"x{t}", name=f"x{t}")
                nc.sync.dma_start(out=xx[:], in_=xr_v[:, t - 1, :, :])
                xt[t] = xx[:]

            # dummy exp: forces the ACT table-set load during the DMA phase
            nc.scalar.activation(warm_act[:], warm_sb[:, 0:8], EXP, scale=1.0)

            # ---- small constants on the gpsimd SWDGE queue ----
            mask_t = constp.tile([128, 128], bf16, tag="mask", name="mask_t")
            nc.gpsimd.dma_start(out=mask_t[:], in_=mask_d[:])
            idb_t = constp.tile([128, 128], bf16, tag="idb", name="idb_t")
            nc.gpsimd.dma_start(out=idb_t[:], in_=idb_d[:])
            idf_t = constp.tile([128, 128], f32, tag="idf", name="idf_t")
            nc.gpsimd.dma_start(out=idf_t[:], in_=idf_d[:])

            qk2 = {}   # [128, TCH] bf16 per chunk: qT (dup'd for chunks 2-3)
            kk2 = {}   # [128, TCH] bf16 per chunk: kT dup'd on both halves
            v1 = {}    # [128, 65] bf16 per s-tile: [v | 1]

            def qk_steps(tch):
                """Emission thunks for chunk `tch`'s q/k projection."""
                steps = []
                state = {}

                def qk_mm(c):
                    def f():
                        if c == 0:
                            state["S"] = miscp.tile(
                                [128, TCH], f32, tag="misc", name=f"Sqk{tch}"
                            )
                        nc.tensor.matmul(
                            state["S"][:, :],
                            wqk_t[:, c, :],
                            xt[tch][:, c, :],
                            start=(c == 0),
                            stop=(c == N_CT - 1),
                            skip_group_check=True,
                        )
                    return f

                def qk_out():
                    S = state["S"]
                    q2 = q2p.tile([128, TCH], bf16, tag="q2", name=f"q2_{tch}")
                    k2 = qkp.tile([128, TCH], bf16, tag=f"k2_{tch}", name=f"k2_{tch}")
                    # q first: the chunk's own scores need it immediately
                    nc.vector.tensor_copy(q2[0:64, :], S[0:64, :])
                    nc.vector.tensor_copy(k2[0:64, :], S[64:128, :])
                    # chunk 0 scores unpacked on half 0 (no dup lead time);
                    # chunks 1-3 pack, their dups have a full window to land.
                    if tch > 0:
                        nc.gpsimd.dma_start(out=q2[64:128, :], in_=q2[0:64, :])
                    nc.gpsimd.dma_start(out=k2[64:128, :], in_=k2[0:64, :])
                    qk2[tch] = q2
                    kk2[tch] = k2

                for c in range(N_CT):
                    steps.append(qk_mm(c))
                steps.append(qk_out)
                return steps

            def v1_build(state, which, tch, i):
                def f():
                    j = 4 * tch + i
                    Pt = miscp.tile([128, H], bf16, tag="misc", name=f"Pt{j}")
                    if which == 0:
                        nc.tensor.transpose(
                            Pt[:, :],
                            state["vTa"][:, 128 * i : 128 * (i + 1)],
                            idb_t[0:64, 0:64],
                        )
                    else:
                        nc.tensor.transpose(
                            Pt[:, :],
                            state["vTb"][64:128, 128 * i : 128 * (i + 1)],
                            idb_t[64:128, 64:128],
                        )
                    v1t = v1p.tile([128, 65], bf16, tag=f"v1_{j}", name=f"v1_{j}")
                    nc.vector.tensor_copy(v1t[:, 0:64], Pt[:, :])
                    nc.vector.memset(v1t[:, 64:65], 1.0)
                    v1[j] = v1t
                return f

            def vpair_steps(ca, cb):
                """v projections for chunks ca/cb column-packed 2x on the PE
                (ca in PE column group 0 -> PSUM partitions 0:64, cb in
                column group 1 -> partitions 64:128), plus the [v|1] builds.
                Returns (head_steps, buildB_steps): buildB (chunk cb's v1)
                has a later deadline."""
                head = []
                state = {}

                def v_mm(c):
                    def f():
                        if c == 0:
                            state["PvA"] = miscp.tile(
                                [64, TCH], f32, tag="misc", name=f"PvA{ca}"
                            )
                            state["PvB"] = miscp.tile(
                                [128, TCH], f32, tag="misc", name=f"PvB{cb}"
                            )
                        nc.tensor.matmul(
                            state["PvA"][:, :],
                            wv_t[:, c, :],
                            xt[ca][:, c, :],
                            start=(c == 0),
                            stop=(c == N_CT - 1),
                            skip_group_check=True,
                        )
                        nc.tensor.matmul(
                            state["PvB"][64:128, :],
                            wv_t[:, c, :],
                            xt[cb][:, c, :],
                            start=(c == 0),
                            stop=(c == N_CT - 1),
                            skip_group_check=True,
                        )
                    return f

                def v_out():
                    vTa = vTp.tile([64, TCH], bf16, tag="vTa", name=f"vT{ca}")
                    nc.vector.tensor_copy(vTa[:], state["PvA"][:, :])
                    vTb = vTp.tile([128, TCH], bf16, tag="vTb", name=f"vT{cb}")
                    nc.vector.tensor_copy(vTb[64:128, :], state["PvB"][64:128, :])
                    state["vTa"] = vTa
                    state["vTb"] = vTb

                for c in range(N_CT):
                    head.append(v_mm(c))
                head.append(v_out)
                for i in range(4):
                    head.append(v1_build(state, 0, ca, i))
                buildB = [v1_build(state, 1, cb, i) for i in range(4)]
                return head, buildB

            def emit_scores_exp(tch, jp):
                """Scores matmuls + exp + masks for pair (jp, jp+1)."""
                unpacked = tch == 0
                S2 = Sp.tile([128, 2 * TCH], f32, tag="S", name=f"S{tch}_{jp}")
                los = {}
                for jj in range(2):
                    j = jp + jj
                    rel = j - 4 * tch
                    lo = 128 * max(0, rel)
                    los[jj] = lo
                    half = slice(0, 64) if unpacked else slice(64 * jj, 64 * (jj + 1))
                    ksl = kk2[j // 4][half, 128 * (j % 4) : 128 * (j % 4 + 1)]
                    nc.tensor.matmul(
                        S2[:, TCH * jj + lo : TCH * (jj + 1)],
                        ksl,
                        qk2[tch][half, lo:TCH],
                        start=True,
                        stop=True,
                        skip_group_check=True,
                    )
                ext = expp.tile([128, 2 * TCH], bf16, tag="ex", name=f"ex{tch}_{jp}")
                nc.scalar.activation(ext[:], S2[:], EXP, scale=SCALE)
                return ext, los

            def emit_pv(tch, jp, ext, los):
                # causal 0/1 mask on the diagonal 128x128 blocks (VectorE),
                # deferred to PV-emission time so it never delays the q/k
                # copies in the DVE stream
                for jj in range(2):
                    if jp + jj - 4 * tch >= 0:
                        a = TCH * jj + los[jj]
                        nc.vector.tensor_mul(
                            ext[:, a : a + 128], ext[:, a : a + 128], mask_t[:]
                        )
                jmax = 4 * tch + 3
                for jj in range(2):
                    j = jp + jj
                    lo = los[jj]
                    nc.tensor.matmul(
                        accs[tch][:, lo:TCH] if j > 0 else accs[tch][:, :],
                        v1[j][:],
                        ext[:, TCH * jj + lo : TCH * (jj + 1)],
                        start=(j == 0),
                        stop=(j == jmax),
                        skip_group_check=True,
                    )

            epi_state = {}

            def emit_epilogue(tch, half=None):
                # ======== normalize + transpose + DMA out for chunk tch ====
                # half=0/1 processes 256 columns (last chunk: gated on the
                # partial PV coverage so the tail pipeline starts early)
                if half in (None, 0):
                    oT = epip.tile([65, TCH], f32, tag="oT", name=f"oT{tch}")
                    ot = epip.tile([128, 4, H], f32, tag="ot", name=f"ot{tch}")
                    epi_state[tch] = (oT, ot)
                oT, ot = epi_state[tch]
                blocks = range(4) if half is None else range(2 * half, 2 * half + 2)
                csl = slice(0, TCH) if half is None else slice(256 * half, 256 * half + 256)
                nc.vector.tensor_copy(oT[:, csl], accs[tch][:, csl])
                for i in blocks:
                    Pe = miscp.tile([128, 65], f32, tag="misc", name=f"Pe{tch}_{i}")
                    nc.tensor.transpose(
                        Pe[:, :],
                        oT[:, 128 * i : 128 * (i + 1)],
                        idf_t[0:65, 0:65],
                    )
                    rec = epip.tile([128, 1], f32, tag="rec", name=f"rec{tch}_{i}")
                    nc.vector.reciprocal(rec[:], Pe[:, 64:65])
                    nc.vector.tensor_scalar_mul(ot[:, i, :], Pe[:, 0:64], rec[:])
                r0 = TCH * tch + (0 if half in (None, 0) else 256)
                nrow = TCH if half is None else 256
                isl = slice(0, 4) if half is None else slice(2 * half, 2 * half + 2)
                nc.sync.dma_start(
                    out=out_d[r0 : r0 + nrow, :].rearrange("(i p) h -> p i h", i=2 if half is not None else 4),
                    in_=ot[:, isl, :],
                )

            # ---- the global pair stream ----
            slots = [
                (tch, jp) for tch in range(N_CHUNK) for jp in range(0, 4 * tch + 4, 2)
            ]
            slot_of = {p: k for k, p in enumerate(slots)}
            n_slots = len(slots)
            # paced projection queues with due slots (emitted between pair
            # slots; ordered dict so earlier-due queues pop first)
            for s in qk_steps(0):
                s()
            v01_head, v01_bB = vpair_steps(0, 1)
            v23_head, v23_bB = vpair_steps(2, 3)
            queues = {
                "qk1": [slot_of[(1, 0)], qk_steps(1)],
                "v01h": [slot_of[(0, 0)] + LAG, v01_head],
                "qk2": [slot_of[(2, 0)] - 1, qk_steps(2)],
                "v01b": [min(slot_of[(1, 4)] + LAG, n_slots), v01_bB],
                "qk3": [slot_of[(3, 0)] - 1, qk_steps(3)],
                "v23h": [min(slot_of[(2, 8)] + LAG, n_slots), v23_head],
                "v23b": [min(slot_of[(3, 12)] + LAG, n_slots), v23_bB],
            }

            accs = {
                tch: accp.tile([65, TCH], f32, tag="acc", name=f"acc{tch}")
                for tch in range(N_CHUNK)
            }

            exts = {}

            blocked_by = {"v01b": "v01h", "v23b": "v23h"}

            def drain_due(k):
                for name, q in queues.items():
                    due, items = q
                    if not items or due - k > 8:  # not urgent yet
                        continue
                    dep = blocked_by.get(name)
                    if dep and queues[dep][1]:  # build needs its head first
                        continue
                    left = max(1, due - k)
                    n = -(-len(items) // left)
                    for _ in range(n):
                        if items:
                            items.pop(0)()

            def force_drain(*names):
                for name in names:
                    items = queues[name][1]
                    while items:
                        items.pop(0)()

            def emit_pv_slot(kv):
                vt, vjp = slots[kv]
                if vjp >= 4 * vt:  # diagonal pair: needs own chunk's v1
                    if vt == 0:
                        force_drain("v01h")
                    elif vt == 1:
                        force_drain("v01h", "v01b")
                    elif vt == 2:
                        force_drain("v23h")
                    else:
                        force_drain("v23h", "v23b")
                emit_pv(vt, vjp, *exts.pop(kv))
                if vt == 3:
                    # last chunk: cols [0:256) are final after pair (3,12),
                    # the rest after (3,14) -> pipeline the epilogue halves
                    if vjp == 12:
                        emit_epilogue(3, half=0)
                    elif vjp == 14:
                        emit_epilogue(3, half=1)
                elif vjp == 4 * vt + 2:  # last pair of chunk vt
                    emit_epilogue(vt)

            for k, (tch, jp) in enumerate(slots):
                # hard guard: chunk's qk projection before its first scores
                if jp == 0 and tch >= 1:
                    force_drain(f"qk{tch}")
                exts[k] = emit_scores_exp(tch, jp)
                if k == 0:
                    # keep the first two score pairs adjacent; no pops yet
                    continue
                if k - LAG >= 0:
                    emit_pv_slot(k - LAG)
                drain_due(k)

            # trailing PVs + last epilogue
            for kv in range(n_slots - LAG, n_slots):
                emit_pv_slot(kv)

    nc.compile()
    return nc


def _get_nc():
    if "nc" not in _CACHE:
        _CACHE["nc"] = _build()
    return _CACHE["nc"]


def _tile_w(w):
    """[C, F] -> [128, N_CT*F] with c-tile-major column blocks."""
    Cdim, F = w.shape
    return np.ascontiguousarray(
        w.reshape(Cdim // 128, 128, F).transpose(1, 0, 2).reshape(128, -1)
    )


def _host_inputs(x, w_q, w_k, w_v):
    bf = ml_dtypes.bfloat16
    x = np.asarray(x, dtype=np.float32)
    wqk = np.concatenate(
        [np.asarray(w_q, np.float32), np.asarray(w_k, np.float32)], 1
    )
    wv = np.asarray(w_v, np.float32)
    wqk_tiled = _tile_w(wqk).astype(bf)
    wv_tiled = _tile_w(wv).astype(bf)
    # multiplicative causal mask for transposed-score diag blocks: keep s <= t
    mask01 = np.triu(np.ones((128, 128), np.float32)).astype(bf)
    idf = np.eye(128, dtype=np.float32)
    idb = np.eye(128, dtype=np.float32).astype(bf)
    in_maps = []
    for i in range(N_CORES):
        # x^T pre-tiled: [128, chunk, c-tile, t] flattened per partition
        xT = np.ascontiguousarray(x[i].T).astype(bf)  # [C, T]
        xT4 = xT.reshape(N_CT, 128, N_CHUNK, TCH)     # [c, p, chunk, t]
        xTt = xT4.transpose(1, 2, 0, 3).reshape(128, N_CHUNK, -1)  # [p, chunk, c*t]
        blob0 = np.ascontiguousarray(
            np.concatenate([wv_tiled, wqk_tiled, xTt[:, 0, :]], axis=1)
        )
        xrest = np.ascontiguousarray(xTt[:, 1:, :].reshape(128, -1))
        in_maps.append(
            {
                "blob0": blob0,
                "xrest": xrest,
                "mask01": mask01,
                "idf": idf,
                "idb": idb,
            }
        )
    return in_maps


def run(x, w_q, w_k, w_v, trace=False, **trace_kwargs):
    from concourse.bass_utils import run_bass_kernel_spmd

    nc = _get_nc()
    in_maps = _host_inputs(x, w_q, w_k, w_v)
    res = run_bass_kernel_spmd(
        nc, in_maps, core_ids=list(range(N_CORES)), trace=trace, **trace_kwargs
    )
    out = np.stack([np.asarray(res.results[i]["out"]) for i in range(N_CORES)])
    return out.astype(np.float32), res


def kernel(x, w_q, w_k, w_v):
    out, _ = run(x, w_q, w_k, w_v, trace=False)
    return out


# revision 26
# speedup vs baseline: 1.3973x; 1.0285x over previous
"""Distributed Trainium2 kernel for a single attention head.

Problem: x:[8,2048,1024] f32, w_q/w_k/w_v:[1024,64] f32
  q,k,v = x@w ; scores = (q k^T)/sqrt(1024) causal-masked; out = softmax(scores)@v

Sharding: data-parallel over batch B=8 across the 8 NeuronCores (one batch
element per core, weights replicated, no collectives).

Per-core dataflow (T=2048, C=1024, H=64), built around keeping the ScalarE
exp stream (the serial resource: 20 x ~1.1us activations) gap-free:

  - host ships blob0 = [wv | wqk | x^T chunk 0] bf16 as ONE DMA at the head
    of the sync HWDGE queue, then x^T chunks 1-3 (one DMA each; HBM is
    shared with the sibling core, so the head transfer is kept minimal).
    q/k partition-duplication DMAs ride the gpsimd SWDGE queue; output DMAs
    ride sync behind the input stream.
  - 16 N=512 warm-up matmuls bridge the PE from the preamble to the blob0
    arrival so the HAM clock-gate is at 8/8 when real work starts.
  - projections with weights stationary (bf16): qkT [128,T] (q rows 0:64,
    k rows 64:128) and vT [64,T] per chunk.
  - scores computed TRANSPOSED per s-tile pair: S[s,t] = kT_slice.T @ qT
    (K=64), written to f32 PSUM tiles [128,1024] (two banks each, Sp
    bufs=2), so two score pairs can be banked ahead of the exp stream.
  - chunks 0-1 run scores unpacked on partition half 0 (no dup round-trip on
    the critical path); chunks 2-3 row-packed 2x in PE row-groups 0/1.
  - exp on ScalarE, scale=1/32 folded in (|scores|<~2, no max needed), one
    [128,1024] activation per pair -> bf16 SBUF; table set pre-loaded by a
    dummy exp during the DMA phase.
  - causal: diagonal 128x128 blocks multiplied by a 0/1 mask on VectorE
    after the exp (keeps the PE free of mask matmuls).
  - PV: out^T[h,t] accumulated per chunk with lhsT = [v | 1] so row 64 is
    the softmax denominator (fused row-sum). PV for pair k is emitted LAG=4
    slots behind its scores, so the in-order PE stream never blocks scores
    production on exp completion.
  - projection work for later chunks is paced between pair slots with
    explicit due-dates (qk of chunk c before its first scores; v of chunk c
    before its first diagonal PV).
  - epilogue: TensorE transpose back to [t,h], reciprocal-multiply on
    VectorE, one combined [512,64] DMA out per chunk.
"""

import os
import sys

import numpy as np

for p in ("/opt/trn_rl_repo",):
    if p not in sys.path and os.path.isdir(p):
        sys.path.insert(0, p)

import ml_dtypes  # noqa: E402

B, T, C, H = 8, 2048, 1024, 64
N_CORES = 8
TCH = 512                  # t-chunk
N_CHUNK = T // TCH         # 4
N_CT = C // 128            # 8 contraction tiles
SCALE = float(C) ** -0.5   # 1/32
N_WARM = 11                # PE warm-up matmuls (N=512 each, ~0.43us cold)
LAG = 4                    # PV trails scores by this many pair slots

_CACHE = {}


def _build():
    """Build + compile the SPMD Bass graph (same graph on all 8 cores)."""
    import concourse.bass as bass
    import concourse.mybir as mybir
    import concourse.tile as tile
    from concourse import bacc

    f32 = mybir.dt.float32
    bf16 = mybir.dt.bfloat16
    EXP = mybir.ActivationFunctionType.Exp

    nc = bacc.Bacc(
        "TRN2", target_bir_lowering=False, debug=False, num_devices=N_CORES
    )

    # blob0 = [wv | wqk | x^T chunk0 | x^T chunk1], x pre-tiled [c-tile][t]
    W_WV = N_CT * H
    W_WQK = N_CT * 128
    W_X = N_CT * TCH
    BLOB0_W = W_WV + W_WQK + W_X
    blob0_d = nc.dram_tensor("blob0", [128, BLOB0_W], bf16, kind="ExternalInput")
    xrest_d = nc.dram_tensor("xrest", [128, 3 * W_X], bf16, kind="ExternalInput")
    mask_d = nc.dram_tensor("mask01", [128, 128], bf16, kind="ExternalInput")
    idf_d = nc.dram_tensor("idf", [128, 128], f32, kind="ExternalInput")
    idb_d = nc.dram_tensor("idb", [128, 128], bf16, kind="ExternalInput")
    out_d = nc.dram_tensor("out", [T, H], f32, kind="ExternalOutput")

    with tile.TileContext(nc) as tc:
        with (
            tc.tile_pool(name="const", bufs=1) as constp,
            tc.tile_pool(name="xTp", bufs=1) as xTp,
            tc.tile_pool(name="qkp", bufs=1) as qkp,
            tc.tile_pool(name="q2p", bufs=2) as q2p,
            tc.tile_pool(name="vTp", bufs=2) as vTp,
            tc.tile_pool(name="v1p", bufs=1) as v1p,
            tc.tile_pool(name="exp", bufs=LAG + 2) as expp,
            tc.tile_pool(name="epi", bufs=2) as epip,
            tc.tile_pool(name="Sp", bufs=2, space="PSUM") as Sp,
            tc.tile_pool(name="accp", bufs=1, space="PSUM") as accp,
            tc.tile_pool(name="miscp", bufs=3, space="PSUM") as miscp,
        ):
            # ---- PE warm-up scratch ----
            warm_sb = constp.tile([128, TCH], bf16, tag="warm_sb", name="warm_sb")
            nc.vector.memset(warm_sb[:], 0.0)
            warm_act = constp.tile([128, 8], bf16, tag="warm_act", name="warm_act")
            warm_ps = miscp.tile([128, TCH], f32, tag="misc", name="warm_ps")
            for i in range(N_WARM):
                nc.tensor.matmul(
                    warm_ps[:, :],
                    warm_sb[:, 0:128],
                    warm_sb[:, :],
                    start=True,
                    stop=True,
                    skip_group_check=True,
                )

            # ---- input DMAs: all on the sync HWDGE queue, strict FIFO ----
            blob0_t = constp.tile([128, BLOB0_W], bf16, tag="blob0", name="blob0_t")
            W_HDR = W_WV + W_WQK
            cuts = [0] + [W_HDR + (W_X * h) // 4 for h in range(1, 4)] + [BLOB0_W]
            for lo, hi in zip(cuts[:-1], cuts[1:]):
                nc.sync.dma_start(out=blob0_t[:, lo:hi], in_=blob0_d[:, lo:hi])
            wv_t = blob0_t[:, 0:W_WV].rearrange("p (n m) -> p n m", n=N_CT)
            wqk_t = blob0_t[:, W_WV : W_WV + W_WQK].rearrange(
                "p (n m) -> p n m", n=N_CT
            )
            a = W_WV + W_WQK
            xt = {0: blob0_t[:, a : a + W_X].rearrange("p (n m) -> p n m", n=N_CT)}
            xr_v = xrest_d[:].rearrange("p (t n m) -> p t n m", t=3, n=N_CT)
            for t in (1, 2, 3):
                xx = xTp.tile([128, N_CT, TCH], bf16, tag=f"x{t}", name=f"x{t}")
                nc.sync.dma_start(out=xx[:], in_=xr_v[:, t - 1, :, :])
                xt[t] = xx[:]

            # dummy exp: forces the ACT table-set load during the DMA phase
            nc.scalar.activation(warm_act[:], warm_sb[:, 0:8], EXP, scale=1.0)

            # ---- small constants on the gpsimd SWDGE queue ----
            mask_t = constp.tile([128, 128], bf16, tag="mask", name="mask_t")
            nc.gpsimd.dma_start(out=mask_t[:], in_=mask_d[:])
            idb_t = constp.tile([128, 128], bf16, tag="idb", name="idb_t")
            nc.gpsimd.dma_start(out=idb_t[:], in_=idb_d[:])
            idf_t = constp.tile([128, 128], f32, tag="idf", name="idf_t")
            nc.gpsimd.dma_start(out=idf_t[:], in_=idf_d[:])

            qk2 = {}   # [128, TCH] bf16 per chunk: qT (dup'd for chunks 2-3)
            kk2 = {}   # [128, TCH] bf16 per chunk: kT dup'd on both halves
            v1 = {}    # [128, 65] bf16 per s-tile: [v | 1]

            def qk_steps(tch):
                """Emission thunks for chunk `tch`'s q/k projection."""
                steps = []
                state = {}

                def qk_mm(c):
                    def f():
                        if c == 0:
                            state["S"] = miscp.tile(
                                [128, TCH], f32, tag="misc", name=f"Sqk{tch}"
                            )
                        nc.tensor.matmul(
                            state["S"][:, :],
                            wqk_t[:, c, :],
                            xt[tch][:, c, :],
                            start=(c == 0),
                            stop=(c == N_CT - 1),
                            skip_group_check=True,
                        )
                    return f

                def qk_out():
                    S = state["S"]
                    q2 = q2p.tile([128, TCH], bf16, tag="q2", name=f"q2_{tch}")
                    k2 = qkp.tile([128, TCH], bf16, tag=f"k2_{tch}", name=f"k2_{tch}")
                    # q first: the chunk's own scores need it immediately
                    nc.vector.tensor_copy(q2[0:64, :], S[0:64, :])
                    nc.vector.tensor_copy(k2[0:64, :], S[64:128, :])
                    # chunk 0 scores unpacked on half 0 (no dup lead time);
                    # chunks 1-3 pack, their dups have a full window to land.
                    if tch > 0:
                        nc.gpsimd.dma_start(out=q2[64:128, :], in_=q2[0:64, :])
                    nc.gpsimd.dma_start(out=k2[64:128, :], in_=k2[0:64, :])
                    qk2[tch] = q2
                    kk2[tch] = k2

                for c in range(N_CT):
                    steps.append(qk_mm(c))
                steps.append(qk_out)
                return steps

            def v1_build(state, which, tch, i):
                def f():
                    j = 4 * tch + i
                    Pt = miscp.tile([128, H], bf16, tag="misc", name=f"Pt{j}")
                    if which == 0:
                        nc.tensor.transpose(
                            Pt[:, :],
                            state["vTa"][:, 128 * i : 128 * (i + 1)],
                            idb_t[0:64, 0:64],
                        )
                    else:
                        nc.tensor.transpose(
                            Pt[:, :],
                            state["vTb"][64:128, 128 * i : 128 * (i + 1)],
                            idb_t[64:128, 64:128],
                        )
                    v1t = v1p.tile([128, 65], bf16, tag=f"v1_{j}", name=f"v1_{j}")
                    nc.vector.tensor_copy(v1t[:, 0:64], Pt[:, :])
                    nc.vector.memset(v1t[:, 64:65], 1.0)
                    v1[j] = v1t
                return f

            def vpair_steps(ca, cb):
                """v projections for chunks ca/cb column-packed 2x on the PE
                (ca in PE column group 0 -> PSUM partitions 0:64, cb in
                column group 1 -> partitions 64:128), plus the [v|1] builds.
                Returns (head_steps, buildB_steps): buildB (chunk cb's v1)
                has a later deadline."""
                head = []
                state = {}

                def v_mm(c):
                    def f():
                        if c == 0:
                            state["PvA"] = miscp.tile(
                                [64, TCH], f32, tag="misc", name=f"PvA{ca}"
                            )
                            state["PvB"] = miscp.tile(
                                [128, TCH], f32, tag="misc", name=f"PvB{cb}"
                            )
                        nc.tensor.matmul(
                            state["PvA"][:, :],
                            wv_t[:, c, :],
                            xt[ca][:, c, :],
                            start=(c == 0),
                            stop=(c == N_CT - 1),
                            skip_group_check=True,
                        )
                        nc.tensor.matmul(
                            state["PvB"][64:128, :],
                            wv_t[:, c, :],
                            xt[cb][:, c, :],
                            start=(c == 0),
                            stop=(c == N_CT - 1),
                            skip_group_check=True,
                        )
                    return f

                def v_out():
                    vTa = vTp.tile([64, TCH], bf16, tag="vTa", name=f"vT{ca}")
                    nc.vector.tensor_copy(vTa[:], state["PvA"][:, :])
                    vTb = vTp.tile([128, TCH], bf16, tag="vTb", name=f"vT{cb}")
                    nc.vector.tensor_copy(vTb[64:128, :], state["PvB"][64:128, :])
                    state["vTa"] = vTa
                    state["vTb"] = vTb

                for c in range(N_CT):
                    head.append(v_mm(c))
                head.append(v_out)
                for i in range(4):
                    head.append(v1_build(state, 0, ca, i))
                buildB = [v1_build(state, 1, cb, i) for i in range(4)]
                return head, buildB

            def emit_scores_exp(tch, jp):
                """Scores matmuls + exp + masks for pair (jp, jp+1)."""
                unpacked = tch == 0 or jp == 0
                S2 = Sp.tile([128, 2 * TCH], f32, tag="S", name=f"S{tch}_{jp}")
                los = {}
                for jj in range(2):
                    j = jp + jj
                    rel = j - 4 * tch
                    lo = 128 * max(0, rel)
                    los[jj] = lo
                    half = slice(0, 64) if unpacked else slice(64 * jj, 64 * (jj + 1))
                    ksl = kk2[j // 4][half, 128 * (j % 4) : 128 * (j % 4 + 1)]
                    nc.tensor.matmul(
                        S2[:, TCH * jj + lo : TCH * (jj + 1)],
                        ksl,
                        qk2[tch][half, lo:TCH],
                        start=True,
                        stop=True,
                        skip_group_check=True,
                    )
                ext = expp.tile([128, 2 * TCH], bf16, tag="ex", name=f"ex{tch}_{jp}")
                nc.scalar.activation(ext[:], S2[:], EXP, scale=SCALE)
                return ext, los

            def emit_pv(tch, jp, ext, los):
                # causal 0/1 mask on the diagonal 128x128 blocks (VectorE),
                # deferred to PV-emission time so it never delays the q/k
                # copies in the DVE stream
                for jj in range(2):
                    if jp + jj - 4 * tch >= 0:
                        a = TCH * jj + los[jj]
                        nc.vector.tensor_mul(
                            ext[:, a : a + 128], ext[:, a : a + 128], mask_t[:]
                        )
                jmax = 4 * tch + 3
                for jj in range(2):
                    j = jp + jj
                    lo = los[jj]
                    nc.tensor.matmul(
                        accs[tch][:, lo:TCH] if j > 0 else accs[tch][:, :],
                        v1[j][:],
                        ext[:, TCH * jj + lo : TCH * (jj + 1)],
                        start=(j == 0),
                        stop=(j == jmax),
                        skip_group_check=True,
                    )

            epi_state = {}

            def emit_epilogue(tch, half=None):
                # ======== normalize + transpose + DMA out for chunk tch ====
                # half=0/1 processes 256 columns (last chunk: gated on the
                # partial PV coverage so the tail pipeline starts early)
                if half in (None, 0):
                    oT = epip.tile([65, TCH], f32, tag="oT", name=f"oT{tch}")
                    ot = epip.tile([128, 4, H], f32, tag="ot", name=f"ot{tch}")
                    epi_state[tch] = (oT, ot)
                oT, ot = epi_state[tch]
                blocks = range(4) if half is None else range(2 * half, 2 * half + 2)
                csl = slice(0, TCH) if half is None else slice(256 * half, 256 * half + 256)
                nc.vector.tensor_copy(oT[:, csl], accs[tch][:, csl])
                for i in blocks:
                    Pe = miscp.tile([128, 65], f32, tag="misc", name=f"Pe{tch}_{i}")
                    nc.tensor.transpose(
                        Pe[:, :],
                        oT[:, 128 * i : 128 * (i + 1)],
                        idf_t[0:65, 0:65],
                    )
                    rec = epip.tile([128, 1], f32, tag="rec", name=f"rec{tch}_{i}")
                    nc.vector.reciprocal(rec[:], Pe[:, 64:65])
                    nc.vector.tensor_scalar_mul(ot[:, i, :], Pe[:, 0:64], rec[:])
                r0 = TCH * tch + (0 if half in (None, 0) else 256)
                nrow = TCH if half is None else 256
                isl = slice(0, 4) if half is None else slice(2 * half, 2 * half + 2)
                nc.sync.dma_start(
                    out=out_d[r0 : r0 + nrow, :].rearrange("(i p) h -> p i h", i=2 if half is not None else 4),
                    in_=ot[:, isl, :],
                )

            # ---- the global pair stream ----
            slots = [
                (tch, jp) for tch in range(N_CHUNK) for jp in range(0, 4 * tch + 4, 2)
            ]
            slot_of = {p: k for k, p in enumerate(slots)}
            n_slots = len(slots)
            # paced projection queues with due slots (emitted between pair
            # slots; ordered dict so earlier-due queues pop first)
            for s in qk_steps(0):
                s()
            v01_head, v01_bB = vpair_steps(0, 1)
            v23_head, v23_bB = vpair_steps(2, 3)
            queues = {
                "qk1": [slot_of[(1, 0)], qk_steps(1)],
                "v01h": [slot_of[(0, 0)] + LAG, v01_head],
                "qk2": [slot_of[(2, 0)] - 1, qk_steps(2)],
                "v01b": [min(slot_of[(1, 4)] + LAG, n_slots), v01_bB],
                "qk3": [slot_of[(3, 0)] - 1, qk_steps(3)],
                "v23h": [min(slot_of[(2, 8)] + LAG, n_slots), v23_head],
                "v23b": [min(slot_of[(3, 12)] + LAG, n_slots), v23_bB],
            }

            accs = {
                tch: accp.tile([65, TCH], f32, tag="acc", name=f"acc{tch}")
                for tch in range(N_CHUNK)
            }

            exts = {}

            blocked_by = {"v01b": "v01h", "v23b": "v23h"}

            def drain_due(k):
                for name, q in queues.items():
                    due, items = q
                    if not items or due - k > 8:  # not urgent yet
                        continue
                    dep = blocked_by.get(name)
                    if dep and queues[dep][1]:  # build needs its head first
                        continue
                    left = max(1, due - k)
                    n = -(-len(items) // left)
                    for _ in range(n):
                        if items:
                            items.pop(0)()

            def force_drain(*names):
                for name in names:
                    items = queues[name][1]
                    while items:
                        items.pop(0)()

            def emit_pv_slot(kv):
                vt, vjp = slots[kv]
                if vjp >= 4 * vt:  # diagonal pair: needs own chunk's v1
                    if vt == 0:
                        force_drain("v01h")
                    elif vt == 1:
                        force_drain("v01h", "v01b")
                    elif vt == 2:
                        force_drain("v23h")
                    else:
                        force_drain("v23h", "v23b")
                emit_pv(vt, vjp, *exts.pop(kv))
                if vt == 3:
                    # last chunk: cols [0:256) are final after pair (3,12),
                    # the rest after (3,14) -> pipeline the epilogue halves
                    if vjp == 12:
                        emit_epilogue(3, half=0)
                    elif vjp == 14:
                        emit_epilogue(3, half=1)
                elif vjp == 4 * vt + 2:  # last pair of chunk vt
                    emit_epilogue(vt)

            for k, (tch, jp) in enumerate(slots):
                # hard guard: chunk's qk projection before its first scores
                if jp == 0 and tch >= 1:
                    force_drain(f"qk{tch}")
                exts[k] = emit_scores_exp(tch, jp)
                if k == 0:
                    # keep the first two score pairs adjacent; no pops yet
                    continue
                if k - LAG >= 0:
                    emit_pv_slot(k - LAG)
                drain_due(k)

            # trailing PVs + last epilogue
            for kv in range(n_slots - LAG, n_slots):
                emit_pv_slot(kv)

    nc.compile()
    return nc


def _get_nc():
    if "nc" not in _CACHE:
        _CACHE["nc"] = _build()
    return _CACHE["nc"]


def _tile_w(w):
    """[C, F] -> [128, N_CT*F] with c-tile-major column blocks."""
    Cdim, F = w.shape
    return np.ascontiguousarray(
        w.reshape(Cdim // 128, 128, F).transpose(1, 0, 2).reshape(128, -1)
    )


def _host_inputs(x, w_q, w_k, w_v):
    bf = ml_dtypes.bfloat16
    x = np.asarray(x, dtype=np.float32)
    wqk = np.concatenate(
        [np.asarray(w_q, np.float32), np.asarray(w_k, np.float32)], 1
    )
    wv = np.asarray(w_v, np.float32)
    wqk_tiled = _tile_w(wqk).astype(bf)
    wv_tiled = _tile_w(wv).astype(bf)
    # multiplicative causal mask for transposed-score diag blocks: keep s <= t
    mask01 = np.triu(np.ones((128, 128), np.float32)).astype(bf)
    idf = np.eye(128, dtype=np.float32)
    idb = np.eye(128, dtype=np.float32).astype(bf)
    in_maps = []
    for i in range(N_CORES):
        # x^T pre-tiled: [128, chunk, c-tile, t] flattened per partition
        xT = np.ascontiguousarray(x[i].T).astype(bf)  # [C, T]
        xT4 = xT.reshape(N_CT, 128, N_CHUNK, TCH)     # [c, p, chunk, t]
        xTt = xT4.transpose(1, 2, 0, 3).reshape(128, N_CHUNK, -1)  # [p, chunk, c*t]
        blob0 = np.ascontiguousarray(
            np.concatenate([wv_tiled, wqk_tiled, xTt[:, 0, :]], axis=1)
        )
        xrest = np.ascontiguousarray(xTt[:, 1:, :].reshape(128, -1))
        in_maps.append(
            {
                "blob0": blob0,
                "xrest": xrest,
                "mask01": mask01,
                "idf": idf,
                "idb": idb,
            }
        )
    return in_maps


def run(x, w_q, w_k, w_v, trace=False, **trace_kwargs):
    from concourse.bass_utils import run_bass_kernel_spmd

    nc = _get_nc()
    in_maps = _host_inputs(x, w_q, w_k, w_v)
    res = run_bass_kernel_spmd(
        nc, in_maps, core_ids=list(range(N_CORES)), trace=trace, **trace_kwargs
    )
    out = np.stack([np.asarray(res.results[i]["out"]) for i in range(N_CORES)])
    return out.astype(np.float32), res


def kernel(x, w_q, w_k, w_v):
    out, _ = run(x, w_q, w_k, w_v, trace=False)
    return out
